# revision 5
# baseline (speedup 1.0000x reference)
"""Trainium2 Bass kernel for the MultiLayerPLEMD (moe_routing) problem.

Data-parallel over the batch axis: 16384 samples -> 8 NeuronCores x 2048.
All expert/gate/tower weights are replicated on every core. No collectives.

Network (per sample x[512]):
  14 expert MLPs (512 ->relu 256 ->relu 128): 4 shared + 6 domain + 2x2 task
  2 task gates: softmax(relu(x@gw1+gb1)@gw2+gb2) over 12 experts
                (10 common + 2 task-specific)
  comb[t] = sum_e g[t,e] * expert_e(x)          (per-sample weighted combine)
  out[t]  = sigmoid(relu(comb@ow1+ob1)@ow2+ob2)

On-chip strategy per core (2048 samples = 4 tiles of 512):
  - x is transposed on host to xT[512, B]; all L1 matmuls run feature-major
    (lhsT = W1 128x128 chunk, rhs = xT chunk, N=512 samples).
  - Expert L2 runs sample-major (lhsT = relu'd h chunk [128h,128s]) so expert
    outputs land as [samples, D] which makes the per-sample weighted combine a
    per-partition-scalar op on the vector engine (scalar_tensor_tensor).
  - Gates: logits computed sample-major [s, 12]; softmax along the free dim.
  - Towers: combined tile is PE-transposed back to feature-major; final
    sigmoid is computed as 0.5*tanh(0.5x + 0.5*b)+0.5 so every activation
    (relu/exp/copy/tanh) lives in the single `exp_and_others` ACT table set.
"""

import os
import sys

for _p in ("/opt/trn_rl_repo",):
    if _p not in sys.path and os.path.isdir(_p):
        sys.path.insert(0, _p)

import numpy as np
from contextlib import ExitStack

import concourse.bass as bass
import concourse.mybir as mybir
import concourse.tile as tile
from concourse.bass_utils import run_bass_kernel_spmd
from concourse.masks import make_identity

FP = mybir.dt.float32
AF = mybir.ActivationFunctionType
ALU = mybir.AluOpType

NCORES = 8
B = 16384
BC = B // NCORES          # samples per core
IN, H, D = 512, 256, 128
NEXP = 14                 # 4 shared + 6 domain + 2*2 task experts
NE = 12                   # experts seen by each task gate
T = 2
GH = 64
TH = 64
TILE = 512                # samples per on-chip tile
NTILES = BC // TILE       # 4
SC = TILE // 128          # sample chunks per tile
KC = IN // 128            # contraction chunks for L1
HC = H // 128             # contraction chunks for L2

_CACHE = {}


def _tasks_of_expert(e):
    """(task, gate_column) pairs that expert e feeds."""
    if e < 10:
        return [(0, e), (1, e)]
    if e < 12:
        return [(0, e)]          # task-0 experts -> gate cols 10, 11
    return [(1, e - 2)]          # task-1 experts -> gate cols 10, 11


def _split_excess_waits(nc, cap=1):
    """walrus's per-engine instruction structs carry a single sync-wait
    command; any scheduled instruction with >1 waits fails codegen ("Too many
    sync wait commands").  Move excess waits onto NoOp instructions inserted
    right before the offending instruction on the same engine."""
    for b in nc.m.functions[0].blocks:
        out = []
        changed = False
        for ins in b.instructions:
            si = getattr(ins, "sync_info", None)
            if si is not None and len(si.on_wait) > cap:
                extra = si.on_wait[:-cap]
                for j, w in enumerate(extra):
                    out.append(mybir.InstNoOp(
                        name=f"{ins.name}-wsplit{j}",
                        engine=ins.engine,
                        ins=[], outs=[],
                        sync_info=mybir.SyncInfo(on_wait=[w], on_update=[]),
                    ))
                ins.sync_info = mybir.SyncInfo(
                    on_wait=si.on_wait[-cap:], on_update=si.on_update)
                changed = True
            out.append(ins)
        if changed:
            b.instructions = out


def _build_bass():
    nc = bass.Bass(trn_type="TRN2", target_bir_lowering=False)

    xT = nc.dram_tensor("xT", [IN, BC], FP, kind="ExternalInput")
    w1 = nc.dram_tensor("w1", [128, NEXP * KC * HC * 128], FP, kind="ExternalInput")
    w2 = nc.dram_tensor("w2", [128, NEXP * HC * 128], FP, kind="ExternalInput")
    b1 = nc.dram_tensor("b1", [128, HC * NEXP], FP, kind="ExternalInput")
    b2 = nc.dram_tensor("b2", [1, NEXP * 128], FP, kind="ExternalInput")
    gw1 = nc.dram_tensor("gw1", [128, KC * T * GH], FP, kind="ExternalInput")
    gb1 = nc.dram_tensor("gb1", [GH, T], FP, kind="ExternalInput")
    gw2 = nc.dram_tensor("gw2", [GH, T * NE], FP, kind="ExternalInput")
    gb2 = nc.dram_tensor("gb2", [1, T * NE], FP, kind="ExternalInput")
    ow1 = nc.dram_tensor("ow1", [D, T * TH], FP, kind="ExternalInput")
    ob1 = nc.dram_tensor("ob1", [TH, T], FP, kind="ExternalInput")
    ow2 = nc.dram_tensor("ow2", [TH, T], FP, kind="ExternalInput")
    ob2h = nc.dram_tensor("ob2h", [1, T], FP, kind="ExternalInput")
    out = nc.dram_tensor("out", [1, BC * T], FP, kind="ExternalOutput")

    with tile.TileContext(nc) as tc, ExitStack() as ctx:
        wpool = ctx.enter_context(tc.tile_pool(name="weights", bufs=1))
        xpool = ctx.enter_context(tc.tile_pool(name="x", bufs=2))
        hpool = ctx.enter_context(tc.tile_pool(name="h", bufs=2))
        eopool = ctx.enter_context(tc.tile_pool(name="eo", bufs=3))
        combpool = ctx.enter_context(tc.tile_pool(name="comb", bufs=2))
        gpool = ctx.enter_context(tc.tile_pool(name="g", bufs=2))
        tpool = ctx.enter_context(tc.tile_pool(name="tower", bufs=2))
        opool = ctx.enter_context(tc.tile_pool(name="outrow", bufs=2))
        psA = ctx.enter_context(tc.tile_pool(name="psA", bufs=2, space=bass.MemorySpace.PSUM))
        psB = ctx.enter_context(tc.tile_pool(name="psB", bufs=2, space=bass.MemorySpace.PSUM))
        psC = ctx.enter_context(tc.tile_pool(name="psC", bufs=2, space=bass.MemorySpace.PSUM))
        psD = ctx.enter_context(tc.tile_pool(name="psD", bufs=2, space=bass.MemorySpace.PSUM))

        # ---- resident weights -------------------------------------------
        w1_sb = wpool.tile([128, NEXP * KC * HC * 128], FP)
        nc.sync.dma_start(w1_sb[:], w1[:])
        w2_sb = wpool.tile([128, NEXP * HC * 128], FP)
        nc.sync.dma_start(w2_sb[:], w2[:])
        b1_sb = wpool.tile([128, HC * NEXP], FP)
        nc.sync.dma_start(b1_sb[:], b1[:])
        b2_sb = wpool.tile([1, NEXP * 128], FP)
        nc.sync.dma_start(b2_sb[:], b2[:])
        gw1_sb = wpool.tile([128, KC * T * GH], FP)
        nc.sync.dma_start(gw1_sb[:], gw1[:])
        gb1_sb = wpool.tile([GH, T], FP)
        nc.sync.dma_start(gb1_sb[:], gb1[:])
        gw2_sb = wpool.tile([GH, T * NE], FP)
        nc.sync.dma_start(gw2_sb[:], gw2[:])
        gb2_sb = wpool.tile([1, T * NE], FP)
        nc.sync.dma_start(gb2_sb[:], gb2[:])
        ow1_sb = wpool.tile([D, T * TH], FP)
        nc.sync.dma_start(ow1_sb[:], ow1[:])
        ob1_sb = wpool.tile([TH, T], FP)
        nc.sync.dma_start(ob1_sb[:], ob1[:])
        ow2_sb = wpool.tile([TH, T], FP)
        nc.sync.dma_start(ow2_sb[:], ow2[:])
        ob2h_sb = wpool.tile([1, T], FP)
        nc.sync.dma_start(ob2h_sb[:], ob2h[:])

        ident = wpool.tile([128, 128], FP)
        make_identity(nc, ident[:])
        ones_sb = wpool.tile([1, 128], FP)
        nc.vector.memset(ones_sb[:], 1.0)

        def w1_sl(e, kc, hc):
            o = ((e * KC + kc) * HC + hc) * 128
            return w1_sb[:, o:o + 128]

        def w2_sl(e, hc):
            o = (e * HC + hc) * 128
            return w2_sb[:, o:o + 128]

        # ---- per-tile pipeline ------------------------------------------
        for i in range(NTILES):
            xt = []
            for kc in range(KC):
                t_ = xpool.tile([128, TILE], FP, tag=f"xt{kc}")
                nc.sync.dma_start(
                    t_[:], xT[kc * 128:(kc + 1) * 128, i * TILE:(i + 1) * TILE])
                xt.append(t_)

            # ---------------- gates ----------------
            g1 = []
            for t in range(T):
                gps = psC.tile([GH, TILE], FP, tag="gate")
                for kc in range(KC):
                    o = (kc * T + t) * GH
                    nc.tensor.matmul(gps[:], gw1_sb[:, o:o + GH], xt[kc][:],
                                     start=(kc == 0), stop=(kc == KC - 1))
                g1t = gpool.tile([GH, TILE], FP, tag=f"g1_{t}")
                nc.scalar.activation(g1t[:], gps[:], AF.Relu,
                                     bias=gb1_sb[:, t:t + 1])
                g1.append(g1t)

            lps = psD.tile([128, SC * T * NE], FP, tag="small")
            for sc in range(SC):
                for t in range(T):
                    o = sc * (T * NE) + t * NE
                    sl = lps[:, o:o + NE]
                    nc.tensor.matmul(sl, ones_sb[:], gb2_sb[:, t * NE:(t + 1) * NE],
                                     start=True, stop=False)
                    nc.tensor.matmul(sl, g1[t][:, sc * 128:(sc + 1) * 128],
                                     gw2_sb[:, t * NE:(t + 1) * NE],
                                     start=False, stop=True)

            exp_sb = gpool.tile([128, SC * T * NE], FP, tag="exp")
            nc.scalar.activation(exp_sb[:], lps[:], AF.Exp)
            sums = gpool.tile([128, SC * T], FP, tag="gsum")
            nc.vector.tensor_reduce(
                sums[:],
                exp_sb[:].rearrange("p (g e) -> p g e", e=NE),
                axis=mybir.AxisListType.X, op=ALU.add)
            rec = gpool.tile([128, SC * T], FP, tag="grec")
            nc.vector.reciprocal(rec[:], sums[:])
            g_sb = gpool.tile([128, SC * T * NE], FP, tag="g")
            for idx in range(SC * T):
                nc.vector.tensor_scalar_mul(
                    g_sb[:, idx * NE:(idx + 1) * NE],
                    exp_sb[:, idx * NE:(idx + 1) * NE],
                    rec[:, idx:idx + 1])

            # ---------------- experts + combine ----------------
            comb = [combpool.tile([128, TILE], FP, tag=f"comb{t}", name=f"comb{t}")
                    for t in range(T)]
            comb_started = [False] * T

            for e in range(NEXP):
                h_tiles = []
                for hc in range(HC):
                    l1ps = psA.tile([128, TILE], FP, tag="l1")
                    for kc in range(KC):
                        nc.tensor.matmul(l1ps[:], w1_sl(e, kc, hc), xt[kc][:],
                                         start=(kc == 0), stop=(kc == KC - 1))
                    ht = hpool.tile([128, TILE], FP, tag=f"h{hc}")
                    nc.scalar.activation(ht[:], l1ps[:], AF.Relu,
                                         bias=b1_sb[:, hc * NEXP + e:hc * NEXP + e + 1])
                    h_tiles.append(ht)

                l2ps = psB.tile([128, TILE], FP, tag="l2")
                for sc in range(SC):
                    sl = l2ps[:, sc * 128:(sc + 1) * 128]
                    nc.tensor.matmul(sl, ones_sb[:], b2_sb[:, e * 128:(e + 1) * 128],
                                     start=True, stop=False)
                    for hc in range(HC):
                        nc.tensor.matmul(sl, h_tiles[hc][:, sc * 128:(sc + 1) * 128],
                                         w2_sl(e, hc),
                                         start=False, stop=(hc == HC - 1))
                eo = eopool.tile([128, TILE], FP, tag="eo")
                nc.scalar.activation(eo[:], l2ps[:], AF.Relu)

                for (t, gcol) in _tasks_of_expert(e):
                    for sc in range(SC):
                        c_sl = comb[t][:, sc * 128:(sc + 1) * 128]
                        e_sl = eo[:, sc * 128:(sc + 1) * 128]
                        g_ap = g_sb[:, sc * (T * NE) + t * NE + gcol:
                                    sc * (T * NE) + t * NE + gcol + 1]
                        if not comb_started[t]:
                            nc.vector.tensor_scalar_mul(c_sl, e_sl, g_ap)
                        else:
                            nc.vector.scalar_tensor_tensor(
                                c_sl, e_sl, g_ap, c_sl, op0=ALU.mult, op1=ALU.add)
                    comb_started[t] = True

            # ---------------- towers ----------------
            orow = opool.tile([1, TILE, T], FP, tag="orow")
            for t in range(T):
                trps = psB.tile([128, TILE], FP, tag="l2")
                for sc in range(SC):
                    nc.tensor.transpose(trps[:, sc * 128:(sc + 1) * 128],
                                        comb[t][:, sc * 128:(sc + 1) * 128],
                                        ident[:])
                combT = tpool.tile([128, TILE], FP, tag="combT")
                nc.scalar.copy(combT[:], trps[:])
                t1ps = psC.tile([TH, TILE], FP, tag="gate")
                nc.tensor.matmul(t1ps[:], ow1_sb[:, t * TH:(t + 1) * TH], combT[:],
                                 start=True, stop=True)
                th = tpool.tile([TH, TILE], FP, tag="th")
                nc.scalar.activation(th[:], t1ps[:], AF.Relu,
                                     bias=ob1_sb[:, t:t + 1])
                t2ps = psD.tile([1, TILE], FP, tag="small")
                nc.tensor.matmul(t2ps[:], ow2_sb[:, t:t + 1], th[:],
                                 start=True, stop=True)
                tnh = opool.tile([1, TILE], FP, tag=f"tanh{t}")
                nc.scalar.activation(tnh[:], t2ps[:], AF.Tanh,
                                     scale=0.5, bias=ob2h_sb[:, t:t + 1])
                # 0.5*tanh + 0.5 = sigmoid, interleaved into [s, T] order
                nc.vector.tensor_scalar(
                    orow[:, :, t], tnh[:], 0.5, 0.5,
                    op0=ALU.mult, op1=ALU.add)

            nc.sync.dma_start(
                out[:, i * TILE * T:(i + 1) * TILE * T],
                orow[:].rearrange("p a b -> p (a b)"))

    _split_excess_waits(nc)
    return nc


def _pack_inputs(x, sw1, sb1, sw2, sb2, dw1, db1, dw2, db2,
                 tw1, tb1, tw2, tb2, gw1, gb1, gw2, gb2,
                 ow1, ob1, ow2, ob2):
    f = np.float32
    w1_all = np.concatenate([sw1, dw1, tw1.reshape(T * 2, IN, H)], 0).astype(f)
    w2_all = np.concatenate([sw2, dw2, tw2.reshape(T * 2, H, D)], 0).astype(f)
    b1_all = np.concatenate([sb1, db1, tb1.reshape(T * 2, H)], 0).astype(f)
    b2_all = np.concatenate([sb2, db2, tb2.reshape(T * 2, D)], 0).astype(f)

    common = {
        "w1": np.ascontiguousarray(
            w1_all.reshape(NEXP, KC, 128, HC, 128).transpose(2, 0, 1, 3, 4)
            .reshape(128, NEXP * KC * HC * 128)),
        "w2": np.ascontiguousarray(
            w2_all.reshape(NEXP, HC, 128, 128).transpose(2, 0, 1, 3)
            .reshape(128, NEXP * HC * 128)),
        "b1": np.ascontiguousarray(
            b1_all.reshape(NEXP, HC, 128).transpose(2, 1, 0).reshape(128, HC * NEXP)),
        "b2": np.ascontiguousarray(b2_all.reshape(1, NEXP * 128)),
        "gw1": np.ascontiguousarray(
            gw1.reshape(T, KC, 128, GH).transpose(2, 1, 0, 3)
            .reshape(128, KC * T * GH)),
        "gb1": np.ascontiguousarray(gb1.T.astype(f)),
        "gw2": np.ascontiguousarray(
            gw2.transpose(1, 0, 2).reshape(GH, T * NE).astype(f)),
        "gb2": np.ascontiguousarray(gb2.reshape(1, T * NE).astype(f)),
        "ow1": np.ascontiguousarray(
            ow1.transpose(1, 0, 2).reshape(D, T * TH).astype(f)),
        "ob1": np.ascontiguousarray(ob1.T.astype(f)),
        "ow2": np.ascontiguousarray(
            ow2.transpose(1, 0, 2).reshape(TH, T).astype(f)),
        "ob2h": np.ascontiguousarray((0.5 * ob2).reshape(1, T).astype(f)),
    }

    xTfull = np.ascontiguousarray(x.astype(f).T)          # [IN, B]
    in_maps = []
    for c in range(NCORES):
        m = dict(common)
        m["xT"] = np.ascontiguousarray(xTfull[:, c * BC:(c + 1) * BC])
        in_maps.append(m)
    return in_maps


def kernel(**inputs):
    inputs = {k: np.asarray(v) for k, v in inputs.items()}
    inputs.pop("domain_ids", None)   # unused by the reference computation
    x = inputs.pop("x")

    if "nc" not in _CACHE:
        _CACHE["nc"] = _build_bass()
    nc = _CACHE["nc"]

    in_maps = _pack_inputs(x=x, **inputs)

    trace = bool(int(os.environ.get("KERNEL_TRACE", "0")))
    res = run_bass_kernel_spmd(nc, in_maps, core_ids=list(range(NCORES)),
                               trace=trace)
    _CACHE["last_results"] = res

    out = np.concatenate(
        [res.results[c]["out"].reshape(BC, T) for c in range(NCORES)], axis=0)
    return out.astype(np.float32)


# revision 6
# speedup vs baseline: 3.0510x; 3.0510x over previous
"""Trainium2 Bass kernel for the MultiLayerPLEMD (moe_routing) problem.

Data-parallel over the batch axis: 16384 samples -> 8 NeuronCores x 2048.
All expert/gate/tower weights are replicated on every core. No collectives.

Network (per sample x[512]):
  14 expert MLPs (512 ->relu 256 ->relu 128): 4 shared + 6 domain + 2x2 task
  2 task gates: softmax(relu(x@gw1+gb1)@gw2+gb2) over 12 experts
                (10 common + 2 task-specific)
  comb[t] = sum_e g[t,e] * expert_e(x)          (per-sample weighted combine)
  out[t]  = sigmoid(relu(comb@ow1+ob1)@ow2+ob2)

On-chip strategy per core (2048 samples = 4 tiles of 512):
  - x is transposed on host to xT[512, B]; all L1 matmuls run feature-major
    (lhsT = W1 128x128 chunk, rhs = xT chunk, N=512 samples).
  - Expert L2 runs sample-major (lhsT = relu'd h chunk [128h,128s]) so expert
    outputs land as [samples, D] which makes the per-sample weighted combine a
    per-partition-scalar op on the vector engine (scalar_tensor_tensor).
  - Gates: logits computed sample-major [s, 12]; softmax along the free dim.
  - Towers: combined tile is PE-transposed back to feature-major; final
    sigmoid is computed as 0.5*tanh(0.5x + 0.5*b)+0.5 so every activation
    (relu/exp/copy/tanh) lives in the single `exp_and_others` ACT table set.
  - Matmul operands are bf16 (fp32 matmuls lower to TWO array passes on
    trn2); accumulation in PSUM and all softmax/bias/combine math is fp32.
"""

import os
import sys

for _p in ("/opt/trn_rl_repo",):
    if _p not in sys.path and os.path.isdir(_p):
        sys.path.insert(0, _p)

import numpy as np
import ml_dtypes
from contextlib import ExitStack

import concourse.bass as bass
import concourse.mybir as mybir
import concourse.tile as tile
from concourse.bass_utils import run_bass_kernel_spmd
from concourse.masks import make_identity

FP = mybir.dt.float32
BF = mybir.dt.bfloat16
NPBF = ml_dtypes.bfloat16
AF = mybir.ActivationFunctionType
ALU = mybir.AluOpType

NCORES = 8
B = 16384
BC = B // NCORES          # samples per core
IN, H, D = 512, 256, 128
NEXP = 14                 # 4 shared + 6 domain + 2*2 task experts
NE = 12                   # experts seen by each task gate
T = 2
GH = 64
TH = 64
TILE = 512                # samples per on-chip tile
NTILES = BC // TILE       # 4
SC = TILE // 128          # sample chunks per tile
KC = IN // 128            # contraction chunks for L1
HC = H // 128             # contraction chunks for L2

_CACHE = {}


def _tasks_of_expert(e):
    """(task, gate_column) pairs that expert e feeds."""
    if e < 10:
        return [(0, e), (1, e)]
    if e < 12:
        return [(0, e)]          # task-0 experts -> gate cols 10, 11
    return [(1, e - 2)]          # task-1 experts -> gate cols 10, 11


def _split_excess_waits(nc, cap=1):
    """walrus's per-engine instruction structs carry a single sync-wait
    command; any scheduled instruction with >1 waits fails codegen ("Too many
    sync wait commands").  Move excess waits onto NoOp instructions inserted
    right before the offending instruction on the same engine."""
    for b in nc.m.functions[0].blocks:
        out = []
        changed = False
        for ins in b.instructions:
            si = getattr(ins, "sync_info", None)
            if si is not None and len(si.on_wait) > cap:
                extra = si.on_wait[:-cap]
                for j, w in enumerate(extra):
                    out.append(mybir.InstNoOp(
                        name=f"{ins.name}-wsplit{j}",
                        engine=ins.engine,
                        ins=[], outs=[],
                        sync_info=mybir.SyncInfo(on_wait=[w], on_update=[]),
                    ))
                ins.sync_info = mybir.SyncInfo(
                    on_wait=si.on_wait[-cap:], on_update=si.on_update)
                changed = True
            out.append(ins)
        if changed:
            b.instructions = out


def _build_bass():
    nc = bass.Bass(trn_type="TRN2", target_bir_lowering=False)

    xT = nc.dram_tensor("xT", [IN, BC], BF, kind="ExternalInput")
    w1 = nc.dram_tensor("w1", [128, NEXP * KC * HC * 128], BF, kind="ExternalInput")
    w2 = nc.dram_tensor("w2", [128, NEXP * HC * 128], BF, kind="ExternalInput")
    b1 = nc.dram_tensor("b1", [128, HC * NEXP], FP, kind="ExternalInput")
    b2 = nc.dram_tensor("b2", [1, NEXP * 128], BF, kind="ExternalInput")
    gw1 = nc.dram_tensor("gw1", [128, KC * T * GH], BF, kind="ExternalInput")
    gb1 = nc.dram_tensor("gb1", [GH, T], FP, kind="ExternalInput")
    gw2 = nc.dram_tensor("gw2", [GH, T * NE], BF, kind="ExternalInput")
    gb2 = nc.dram_tensor("gb2", [1, T * NE], BF, kind="ExternalInput")
    ow1 = nc.dram_tensor("ow1", [D, T * TH], BF, kind="ExternalInput")
    ob1 = nc.dram_tensor("ob1", [TH, T], FP, kind="ExternalInput")
    ow2 = nc.dram_tensor("ow2", [TH, T], BF, kind="ExternalInput")
    ob2h = nc.dram_tensor("ob2h", [1, T], FP, kind="ExternalInput")
    out = nc.dram_tensor("out", [1, BC * T], FP, kind="ExternalOutput")

    with tile.TileContext(nc) as tc, ExitStack() as ctx:
        wpool = ctx.enter_context(tc.tile_pool(name="weights", bufs=1))
        xpool = ctx.enter_context(tc.tile_pool(name="x", bufs=2))
        hpool = ctx.enter_context(tc.tile_pool(name="h", bufs=2))
        eopool = ctx.enter_context(tc.tile_pool(name="eo", bufs=3))
        combpool = ctx.enter_context(tc.tile_pool(name="comb", bufs=2))
        gpool = ctx.enter_context(tc.tile_pool(name="g", bufs=2))
        tpool = ctx.enter_context(tc.tile_pool(name="tower", bufs=2))
        opool = ctx.enter_context(tc.tile_pool(name="outrow", bufs=2))
        psA = ctx.enter_context(tc.tile_pool(name="psA", bufs=2, space=bass.MemorySpace.PSUM))
        psB = ctx.enter_context(tc.tile_pool(name="psB", bufs=2, space=bass.MemorySpace.PSUM))
        psC = ctx.enter_context(tc.tile_pool(name="psC", bufs=2, space=bass.MemorySpace.PSUM))
        psD = ctx.enter_context(tc.tile_pool(name="psD", bufs=2, space=bass.MemorySpace.PSUM))

        # ---- resident weights -------------------------------------------
        w1_sb = wpool.tile([128, NEXP * KC * HC * 128], BF)
        nc.sync.dma_start(w1_sb[:], w1[:])
        w2_sb = wpool.tile([128, NEXP * HC * 128], BF)
        nc.sync.dma_start(w2_sb[:], w2[:])
        b1_sb = wpool.tile([128, HC * NEXP], FP)
        nc.sync.dma_start(b1_sb[:], b1[:])
        b2_sb = wpool.tile([1, NEXP * 128], BF)
        nc.sync.dma_start(b2_sb[:], b2[:])
        gw1_sb = wpool.tile([128, KC * T * GH], BF)
        nc.sync.dma_start(gw1_sb[:], gw1[:])
        gb1_sb = wpool.tile([GH, T], FP)
        nc.sync.dma_start(gb1_sb[:], gb1[:])
        gw2_sb = wpool.tile([GH, T * NE], BF)
        nc.sync.dma_start(gw2_sb[:], gw2[:])
        gb2_sb = wpool.tile([1, T * NE], BF)
        nc.sync.dma_start(gb2_sb[:], gb2[:])
        ow1_sb = wpool.tile([D, T * TH], BF)
        nc.sync.dma_start(ow1_sb[:], ow1[:])
        ob1_sb = wpool.tile([TH, T], FP)
        nc.sync.dma_start(ob1_sb[:], ob1[:])
        ow2_sb = wpool.tile([TH, T], BF)
        nc.sync.dma_start(ow2_sb[:], ow2[:])
        ob2h_sb = wpool.tile([1, T], FP)
        nc.sync.dma_start(ob2h_sb[:], ob2h[:])

        ident = wpool.tile([128, 128], FP)
        make_identity(nc, ident[:])
        ones_sb = wpool.tile([1, 128], BF)
        nc.vector.memset(ones_sb[:], 1.0)

        def w1_sl(e, kc, hc):
            o = ((e * KC + kc) * HC + hc) * 128
            return w1_sb[:, o:o + 128]

        def w2_sl(e, hc):
            o = (e * HC + hc) * 128
            return w2_sb[:, o:o + 128]

        # ---- per-tile pipeline ------------------------------------------
        for i in range(NTILES):
            xt = []
            for kc in range(KC):
                t_ = xpool.tile([128, TILE], BF, tag=f"xt{kc}")
                nc.sync.dma_start(
                    t_[:], xT[kc * 128:(kc + 1) * 128, i * TILE:(i + 1) * TILE])
                xt.append(t_)

            # ---------------- gates ----------------
            g1 = []
            for t in range(T):
                gps = psC.tile([GH, TILE], FP, tag="gate")
                for kc in range(KC):
                    o = (kc * T + t) * GH
                    nc.tensor.matmul(gps[:], gw1_sb[:, o:o + GH], xt[kc][:],
                                     start=(kc == 0), stop=(kc == KC - 1))
                g1t = gpool.tile([GH, TILE], BF, tag=f"g1_{t}")
                nc.scalar.activation(g1t[:], gps[:], AF.Relu,
                                     bias=gb1_sb[:, t:t + 1])
                g1.append(g1t)

            lps = psD.tile([128, SC * T * NE], FP, tag="small")
            for sc in range(SC):
                for t in range(T):
                    o = sc * (T * NE) + t * NE
                    sl = lps[:, o:o + NE]
                    nc.tensor.matmul(sl, ones_sb[:], gb2_sb[:, t * NE:(t + 1) * NE],
                                     start=True, stop=False)
                    nc.tensor.matmul(sl, g1[t][:, sc * 128:(sc + 1) * 128],
                                     gw2_sb[:, t * NE:(t + 1) * NE],
                                     start=False, stop=True)

            exp_sb = gpool.tile([128, SC * T * NE], FP, tag="exp")
            nc.scalar.activation(exp_sb[:], lps[:], AF.Exp)
            sums = gpool.tile([128, SC * T], FP, tag="gsum")
            nc.vector.tensor_reduce(
                sums[:],
                exp_sb[:].rearrange("p (g e) -> p g e", e=NE),
                axis=mybir.AxisListType.X, op=ALU.add)
            rec = gpool.tile([128, SC * T], FP, tag="grec")
            nc.vector.reciprocal(rec[:], sums[:])
            g_sb = gpool.tile([128, SC * T * NE], FP, tag="g")
            for idx in range(SC * T):
                nc.vector.tensor_scalar_mul(
                    g_sb[:, idx * NE:(idx + 1) * NE],
                    exp_sb[:, idx * NE:(idx + 1) * NE],
                    rec[:, idx:idx + 1])

            # ---------------- experts + combine ----------------
            comb = [combpool.tile([128, TILE], FP, tag=f"comb{t}", name=f"comb{t}")
                    for t in range(T)]
            comb_started = [False] * T

            for e in range(NEXP):
                h_tiles = []
                for hc in range(HC):
                    l1ps = psA.tile([128, TILE], FP, tag="l1")
                    for kc in range(KC):
                        nc.tensor.matmul(l1ps[:], w1_sl(e, kc, hc), xt[kc][:],
                                         start=(kc == 0), stop=(kc == KC - 1))
                    ht = hpool.tile([128, TILE], BF, tag=f"h{hc}")
                    nc.scalar.activation(ht[:], l1ps[:], AF.Relu,
                                         bias=b1_sb[:, hc * NEXP + e:hc * NEXP + e + 1])
                    h_tiles.append(ht)

                l2ps = psB.tile([128, TILE], FP, tag="l2")
                for sc in range(SC):
                    sl = l2ps[:, sc * 128:(sc + 1) * 128]
                    nc.tensor.matmul(sl, ones_sb[:], b2_sb[:, e * 128:(e + 1) * 128],
                                     start=True, stop=False)
                    for hc in range(HC):
                        nc.tensor.matmul(sl, h_tiles[hc][:, sc * 128:(sc + 1) * 128],
                                         w2_sl(e, hc),
                                         start=False, stop=(hc == HC - 1))
                eo = eopool.tile([128, TILE], BF, tag="eo")
                nc.scalar.activation(eo[:], l2ps[:], AF.Relu)

                for (t, gcol) in _tasks_of_expert(e):
                    for sc in range(SC):
                        c_sl = comb[t][:, sc * 128:(sc + 1) * 128]
                        e_sl = eo[:, sc * 128:(sc + 1) * 128]
                        g_ap = g_sb[:, sc * (T * NE) + t * NE + gcol:
                                    sc * (T * NE) + t * NE + gcol + 1]
                        if not comb_started[t]:
                            nc.vector.tensor_scalar_mul(c_sl, e_sl, g_ap)
                        else:
                            nc.vector.scalar_tensor_tensor(
                                c_sl, e_sl, g_ap, c_sl, op0=ALU.mult, op1=ALU.add)
                    comb_started[t] = True

            # ---------------- towers ----------------
            orow = opool.tile([1, TILE, T], FP, tag="orow")
            for t in range(T):
                trps = psB.tile([128, TILE], FP, tag="l2")
                for sc in range(SC):
                    nc.tensor.transpose(trps[:, sc * 128:(sc + 1) * 128],
                                        comb[t][:, sc * 128:(sc + 1) * 128],
                                        ident[:])
                combT = tpool.tile([128, TILE], BF, tag="combT")
                nc.scalar.copy(combT[:], trps[:])
                t1ps = psC.tile([TH, TILE], FP, tag="gate")
                nc.tensor.matmul(t1ps[:], ow1_sb[:, t * TH:(t + 1) * TH], combT[:],
                                 start=True, stop=True)
                th = tpool.tile([TH, TILE], BF, tag="th")
                nc.scalar.activation(th[:], t1ps[:], AF.Relu,
                                     bias=ob1_sb[:, t:t + 1])
                t2ps = psD.tile([1, TILE], FP, tag="small")
                nc.tensor.matmul(t2ps[:], ow2_sb[:, t:t + 1], th[:],
                                 start=True, stop=True)
                tnh = opool.tile([1, TILE], FP, tag=f"tanh{t}")
                nc.scalar.activation(tnh[:], t2ps[:], AF.Tanh,
                                     scale=0.5, bias=ob2h_sb[:, t:t + 1])
                # 0.5*tanh + 0.5 = sigmoid, interleaved into [s, T] order
                nc.vector.tensor_scalar(
                    orow[:, :, t], tnh[:], 0.5, 0.5,
                    op0=ALU.mult, op1=ALU.add)

            nc.sync.dma_start(
                out[:, i * TILE * T:(i + 1) * TILE * T],
                orow[:].rearrange("p a b -> p (a b)"))

    _split_excess_waits(nc)
    return nc


def _pack_inputs(x, sw1, sb1, sw2, sb2, dw1, db1, dw2, db2,
                 tw1, tb1, tw2, tb2, gw1, gb1, gw2, gb2,
                 ow1, ob1, ow2, ob2):
    f = np.float32
    w1_all = np.concatenate([sw1, dw1, tw1.reshape(T * 2, IN, H)], 0).astype(f)
    w2_all = np.concatenate([sw2, dw2, tw2.reshape(T * 2, H, D)], 0).astype(f)
    b1_all = np.concatenate([sb1, db1, tb1.reshape(T * 2, H)], 0).astype(f)
    b2_all = np.concatenate([sb2, db2, tb2.reshape(T * 2, D)], 0).astype(f)

    common = {
        "w1": np.ascontiguousarray(
            w1_all.reshape(NEXP, KC, 128, HC, 128).transpose(2, 0, 1, 3, 4)
            .reshape(128, NEXP * KC * HC * 128)).astype(NPBF),
        "w2": np.ascontiguousarray(
            w2_all.reshape(NEXP, HC, 128, 128).transpose(2, 0, 1, 3)
            .reshape(128, NEXP * HC * 128)).astype(NPBF),
        "b1": np.ascontiguousarray(
            b1_all.reshape(NEXP, HC, 128).transpose(2, 1, 0).reshape(128, HC * NEXP)),
        "b2": np.ascontiguousarray(b2_all.reshape(1, NEXP * 128)).astype(NPBF),
        "gw1": np.ascontiguousarray(
            gw1.reshape(T, KC, 128, GH).transpose(2, 1, 0, 3)
            .reshape(128, KC * T * GH)).astype(NPBF),
        "gb1": np.ascontiguousarray(gb1.T.astype(f)),
        "gw2": np.ascontiguousarray(
            gw2.transpose(1, 0, 2).reshape(GH, T * NE)).astype(NPBF),
        "gb2": np.ascontiguousarray(gb2.reshape(1, T * NE)).astype(NPBF),
        "ow1": np.ascontiguousarray(
            ow1.transpose(1, 0, 2).reshape(D, T * TH)).astype(NPBF),
        "ob1": np.ascontiguousarray(ob1.T.astype(f)),
        "ow2": np.ascontiguousarray(
            ow2.transpose(1, 0, 2).reshape(TH, T)).astype(NPBF),
        "ob2h": np.ascontiguousarray((0.5 * ob2).reshape(1, T).astype(f)),
    }

    xTfull = np.ascontiguousarray(x.astype(f).T.astype(NPBF))   # [IN, B]
    in_maps = []
    for c in range(NCORES):
        m = dict(common)
        m["xT"] = np.ascontiguousarray(xTfull[:, c * BC:(c + 1) * BC])
        in_maps.append(m)
    return in_maps


def kernel(**inputs):
    inputs = {k: np.asarray(v) for k, v in inputs.items()}
    inputs.pop("domain_ids", None)   # unused by the reference computation
    x = inputs.pop("x")

    if "nc" not in _CACHE:
        _CACHE["nc"] = _build_bass()
    nc = _CACHE["nc"]

    in_maps = _pack_inputs(x=x, **inputs)

    trace = bool(int(os.environ.get("KERNEL_TRACE", "0")))
    res = run_bass_kernel_spmd(nc, in_maps, core_ids=list(range(NCORES)),
                               trace=trace)
    _CACHE["last_results"] = res

    out = np.concatenate(
        [res.results[c]["out"].reshape(BC, T) for c in range(NCORES)], axis=0)
    return out.astype(np.float32)


# revision 18
# speedup vs baseline: 3.4941x; 1.1452x over previous
"""Trainium2 Bass kernel for the MultiLayerPLEMD (moe_routing) problem.

Data-parallel over the batch axis: 16384 samples -> 8 NeuronCores x 2048.
All expert/gate/tower weights are replicated on every core. No collectives.

Network (per sample x[512]):
  14 expert MLPs (512 ->relu 256 ->relu 128): 4 shared + 6 domain + 2x2 task
  2 task gates: softmax(relu(x@gw1+gb1)@gw2+gb2) over 12 experts
                (10 common + 2 task-specific)
  comb[t] = sum_e g[t,e] * expert_e(x)          (per-sample weighted combine)
  out[t]  = sigmoid(relu(comb@ow1+ob1)@ow2+ob2)

On-chip strategy per core (2048 samples = 4 tiles of 512):
  - x is transposed on host to xT[512, B]; all L1 matmuls run feature-major
    (lhsT = W1 128x128 chunk, rhs = xT chunk, N=512 samples).
  - Expert L2 runs sample-major (lhsT = relu'd h chunk [128h,128s]) so expert
    outputs land as [samples, D] which makes the per-sample weighted combine a
    per-partition-scalar op on the vector engine (scalar_tensor_tensor).
  - Gates: logits computed sample-major [s, 12]; softmax along the free dim.
  - Towers: combined tile is PE-transposed back to feature-major; final
    sigmoid is computed as 0.5*tanh(0.5x + 0.5*b)+0.5 so every activation
    (relu/exp/copy/tanh) lives in the single `exp_and_others` ACT table set.
  - Matmul operands are bf16 (fp32 matmuls lower to TWO array passes on
    trn2); accumulation in PSUM and all softmax/bias/combine math is fp32.
"""

import os
import sys

for _p in ("/opt/trn_rl_repo",):
    if _p not in sys.path and os.path.isdir(_p):
        sys.path.insert(0, _p)

import numpy as np
import ml_dtypes
from contextlib import ExitStack

import concourse.bass as bass
import concourse.mybir as mybir
import concourse.tile as tile
from concourse.bass_utils import run_bass_kernel_spmd
from concourse.masks import make_identity

FP = mybir.dt.float32
BF = mybir.dt.bfloat16
NPBF = ml_dtypes.bfloat16
AF = mybir.ActivationFunctionType
ALU = mybir.AluOpType

NCORES = 8
B = 16384
BC = B // NCORES          # samples per core
IN, H, D = 512, 256, 128
NEXP = 14                 # 4 shared + 6 domain + 2*2 task experts
NE = 12                   # experts seen by each task gate
T = 2
GH = 64
TH = 64
TILE = 512                # samples per on-chip tile
NTILES = BC // TILE       # 4
SC = TILE // 128          # sample chunks per tile
KC = IN // 128            # contraction chunks for L1
HC = H // 128             # contraction chunks for L2

_CACHE = {}
# matmul lhsT/rhs at SBUF partition offset 64 compiles but faults at runtime
# on this toolchain, so gate L2 copies each task's operands to partition 0.
STACK_GATE_L2 = bool(int(os.environ.get("STACK_GATE_L2", "0")))


def _tasks_of_expert(e):
    """(task, gate_column) pairs that expert e feeds."""
    if e < 10:
        return [(0, e), (1, e)]
    if e < 12:
        return [(0, e)]          # task-0 experts -> gate cols 10, 11
    return [(1, e - 2)]          # task-1 experts -> gate cols 10, 11


def _split_excess_waits(nc, cap=1):
    """walrus's per-engine instruction structs carry a single sync-wait
    command; any scheduled instruction with >1 waits fails codegen ("Too many
    sync wait commands").  Move excess waits onto NoOp instructions inserted
    right before the offending instruction on the same engine."""
    for b in nc.m.functions[0].blocks:
        out = []
        changed = False
        for ins in b.instructions:
            si = getattr(ins, "sync_info", None)
            if si is not None and len(si.on_wait) > cap:
                extra = si.on_wait[:-cap]
                for j, w in enumerate(extra):
                    out.append(mybir.InstNoOp(
                        name=f"{ins.name}-wsplit{j}",
                        engine=ins.engine,
                        ins=[], outs=[],
                        sync_info=mybir.SyncInfo(on_wait=[w], on_update=[]),
                    ))
                ins.sync_info = mybir.SyncInfo(
                    on_wait=si.on_wait[-cap:], on_update=si.on_update)
                changed = True
            out.append(ins)
        if changed:
            b.instructions = out


def _build_bass():
    nc = bass.Bass(trn_type="TRN2", target_bir_lowering=False)

    xT = nc.dram_tensor("xT", [IN, BC], BF, kind="ExternalInput")
    w1 = nc.dram_tensor("w1", [128, NEXP * KC * HC * 128], BF, kind="ExternalInput")
    w2 = nc.dram_tensor("w2", [128, NEXP * HC * 128], BF, kind="ExternalInput")
    b1 = nc.dram_tensor("b1", [128, HC * NEXP], FP, kind="ExternalInput")
    b2 = nc.dram_tensor("b2", [1, NEXP * TILE], BF, kind="ExternalInput")
    gw1 = nc.dram_tensor("gw1", [128, KC * T * GH], BF, kind="ExternalInput")
    gb1 = nc.dram_tensor("gb1", [T * GH, 1], FP, kind="ExternalInput")
    gw2 = nc.dram_tensor("gw2", [T * GH, NE], BF, kind="ExternalInput")
    gb2 = nc.dram_tensor("gb2", [1, T * NE], BF, kind="ExternalInput")
    ow1 = nc.dram_tensor("ow1", [D, T * TH], BF, kind="ExternalInput")
    ob1 = nc.dram_tensor("ob1", [TH, T], FP, kind="ExternalInput")
    ow2 = nc.dram_tensor("ow2", [TH, T], BF, kind="ExternalInput")
    ob2h = nc.dram_tensor("ob2h", [1, T], FP, kind="ExternalInput")
    out = nc.dram_tensor("out", [1, BC * T], FP, kind="ExternalOutput")

    with tile.TileContext(nc) as tc, ExitStack() as ctx:
        wpool = ctx.enter_context(tc.tile_pool(name="weights", bufs=1))
        xpool = ctx.enter_context(tc.tile_pool(name="x", bufs=2))
        hpool = ctx.enter_context(tc.tile_pool(name="h", bufs=2))
        eopool = ctx.enter_context(tc.tile_pool(name="eo", bufs=3))
        combpool = ctx.enter_context(tc.tile_pool(name="comb", bufs=2))
        gpool = ctx.enter_context(tc.tile_pool(name="g", bufs=2))
        tpool = ctx.enter_context(tc.tile_pool(name="tower", bufs=2))
        opool = ctx.enter_context(tc.tile_pool(name="outrow", bufs=2))
        psA = ctx.enter_context(tc.tile_pool(name="psA", bufs=2, space=bass.MemorySpace.PSUM))
        psB = ctx.enter_context(tc.tile_pool(name="psB", bufs=2, space=bass.MemorySpace.PSUM))
        psC = ctx.enter_context(tc.tile_pool(name="psC", bufs=2, space=bass.MemorySpace.PSUM))
        psD = ctx.enter_context(tc.tile_pool(name="psD", bufs=2, space=bass.MemorySpace.PSUM))

        # ---- resident weights -------------------------------------------
        # Small gate/tower weights + the first x tile first, so compute can
        # start while the bulk expert weights stream in per-expert.
        gw1_sb = wpool.tile([128, KC * T * GH], BF)
        nc.sync.dma_start(gw1_sb[:], gw1[:])
        gb1_sb = wpool.tile([T * GH, 1], FP)
        nc.sync.dma_start(gb1_sb[:], gb1[:])
        gw2_sb = wpool.tile([T * GH, NE], BF)
        nc.sync.dma_start(gw2_sb[:], gw2[:])
        gb2_sb = wpool.tile([1, T * NE], BF)
        nc.sync.dma_start(gb2_sb[:], gb2[:])
        b1_sb = wpool.tile([128, HC * NEXP], FP)
        nc.sync.dma_start(b1_sb[:], b1[:])
        b2_sb = wpool.tile([1, NEXP * TILE], BF)
        nc.sync.dma_start(b2_sb[:], b2[:])

        w1_sb = wpool.tile([128, NEXP * KC * HC * 128], BF)
        w2_sb = wpool.tile([128, NEXP * HC * 128], BF)
        for e in range(NEXP):
            o = e * KC * HC * 128
            nc.sync.dma_start(w1_sb[:, o:o + KC * HC * 128],
                              w1[:, o:o + KC * HC * 128])
            o2 = e * HC * 128
            nc.sync.dma_start(w2_sb[:, o2:o2 + HC * 128],
                              w2[:, o2:o2 + HC * 128])

        ow1_sb = wpool.tile([D, T * TH], BF)
        nc.sync.dma_start(ow1_sb[:], ow1[:])
        ob1_sb = wpool.tile([TH, T], FP)
        nc.sync.dma_start(ob1_sb[:], ob1[:])
        ow2_sb = wpool.tile([TH, T], BF)
        nc.sync.dma_start(ow2_sb[:], ow2[:])
        ob2h_sb = wpool.tile([1, T], FP)
        nc.sync.dma_start(ob2h_sb[:], ob2h[:])

        ident = wpool.tile([128, 128], FP)
        make_identity(nc, ident[:])
        ones_sb = wpool.tile([1, 128], BF)
        nc.vector.memset(ones_sb[:], 1.0)

        def w1_sl(e, kc, hc):
            o = ((e * KC + kc) * HC + hc) * 128
            return w1_sb[:, o:o + 128]

        def w2_sl(e, hc):
            o = (e * HC + hc) * 128
            return w2_sb[:, o:o + 128]

        # ---- per-tile pipeline ------------------------------------------
        for i in range(NTILES):
            xt = []
            for kc in range(KC):
                t_ = xpool.tile([128, TILE], BF, tag=f"xt{kc}")
                nc.sync.dma_start(
                    t_[:], xT[kc * 128:(kc + 1) * 128, i * TILE:(i + 1) * TILE])
                xt.append(t_)

            # ---------------- gates ----------------
            # both tasks' gate hiddens stacked on the partition axis (2*64)
            gps = psC.tile([T * GH, TILE], FP, tag="gate")
            for kc in range(KC):
                nc.tensor.matmul(gps[:], gw1_sb[:, kc * 128:(kc + 1) * 128],
                                 xt[kc][:],
                                 start=(kc == 0), stop=(kc == KC - 1))
            if STACK_GATE_L2:
                g1 = gpool.tile([T * GH, TILE], BF, tag="g1")
                nc.scalar.activation(g1[:], gps[:], AF.Relu, bias=gb1_sb[:])
                g1s = [g1[t * GH:(t + 1) * GH, :] for t in range(T)]
                gw2s = [gw2_sb[t * GH:(t + 1) * GH, :] for t in range(T)]
            else:
                g1s, gw2s = [], []
                for t in range(T):
                    g1t = gpool.tile([GH, TILE], BF, tag=f"g1_{t}", name=f"g1_{t}")
                    nc.scalar.activation(g1t[:], gps[t * GH:(t + 1) * GH, :],
                                         AF.Relu, bias=gb1_sb[t * GH:(t + 1) * GH, :])
                    g1s.append(g1t[:, :])
                    gw2t = gpool.tile([GH, NE], BF, tag=f"gw2_{t}", name=f"gw2_{t}")
                    nc.vector.tensor_copy(gw2t[:], gw2_sb[t * GH:(t + 1) * GH, :])
                    gw2s.append(gw2t[:, :])

            lps = psD.tile([128, SC * T * NE], FP, tag="small")
            for sc in range(SC):
                o = sc * (T * NE)
                nc.tensor.matmul(lps[:, o:o + T * NE], ones_sb[:], gb2_sb[:],
                                 start=True, stop=False, skip_group_check=True)
                for t in range(T):
                    nc.tensor.matmul(lps[:, o + t * NE:o + (t + 1) * NE],
                                     g1s[t][:, sc * 128:(sc + 1) * 128],
                                     gw2s[t],
                                     start=False, stop=(t == T - 1),
                                     skip_group_check=True)

            exp_sb = gpool.tile([128, SC * T * NE], FP, tag="exp")
            nc.scalar.activation(exp_sb[:], lps[:], AF.Exp)
            sums = gpool.tile([128, SC * T], FP, tag="gsum")
            nc.vector.tensor_reduce(
                sums[:],
                exp_sb[:].rearrange("p (g e) -> p g e", e=NE),
                axis=mybir.AxisListType.X, op=ALU.add)
            rec = gpool.tile([128, SC * T], FP, tag="grec")
            nc.vector.reciprocal(rec[:], sums[:])
            g_sb = gpool.tile([128, SC * T * NE], FP, tag="g")
            for idx in range(SC * T):
                nc.vector.tensor_scalar_mul(
                    g_sb[:, idx * NE:(idx + 1) * NE],
                    exp_sb[:, idx * NE:(idx + 1) * NE],
                    rec[:, idx:idx + 1])

            # ---------------- experts + combine ----------------
            comb = [combpool.tile([128, TILE], FP, tag=f"comb{t}", name=f"comb{t}")
                    for t in range(T)]
            comb_started = [False] * T

            for e in range(NEXP):
                h_tiles = []
                for hc in range(HC):
                    l1ps = psA.tile([128, TILE], FP, tag="l1")
                    for kc in range(KC):
                        nc.tensor.matmul(l1ps[:], w1_sl(e, kc, hc), xt[kc][:],
                                         start=(kc == 0), stop=(kc == KC - 1))
                    ht = hpool.tile([128, TILE], BF, tag=f"h{hc}")
                    nc.scalar.activation(ht[:], l1ps[:], AF.Relu,
                                         bias=b1_sb[:, hc * NEXP + e:hc * NEXP + e + 1])
                    h_tiles.append(ht)

                l2ps = psB.tile([128, TILE], FP, tag="l2")
                nc.tensor.matmul(l2ps[:], ones_sb[:],
                                 b2_sb[:, e * TILE:(e + 1) * TILE],
                                 start=True, stop=False, skip_group_check=True)
                for sc in range(SC):
                    sl = l2ps[:, sc * 128:(sc + 1) * 128]
                    for hc in range(HC):
                        nc.tensor.matmul(sl, h_tiles[hc][:, sc * 128:(sc + 1) * 128],
                                         w2_sl(e, hc),
                                         start=False,
                                         stop=(sc == SC - 1 and hc == HC - 1),
                                         skip_group_check=True)
                eo = eopool.tile([128, TILE], BF, tag="eo")
                nc.scalar.activation(eo[:], l2ps[:], AF.Relu)

                for (t, gcol) in _tasks_of_expert(e):
                    eng = nc.vector
                    for sc in range(SC):
                        c_sl = comb[t][:, sc * 128:(sc + 1) * 128]
                        e_sl = eo[:, sc * 128:(sc + 1) * 128]
                        g_ap = g_sb[:, sc * (T * NE) + t * NE + gcol:
                                    sc * (T * NE) + t * NE + gcol + 1]
                        if not comb_started[t]:
                            eng.tensor_scalar_mul(c_sl, e_sl, g_ap)
                        else:
                            eng.scalar_tensor_tensor(
                                c_sl, e_sl, g_ap, c_sl, op0=ALU.mult, op1=ALU.add)
                    comb_started[t] = True

            # ---------------- towers ----------------
            orow = opool.tile([1, TILE, T], FP, tag="orow")
            for t in range(T):
                trps = psB.tile([128, TILE], FP, tag="l2")
                for sc in range(SC):
                    nc.tensor.transpose(trps[:, sc * 128:(sc + 1) * 128],
                                        comb[t][:, sc * 128:(sc + 1) * 128],
                                        ident[:])
                combT = tpool.tile([128, TILE], BF, tag="combT")
                nc.scalar.copy(combT[:], trps[:])
                t1ps = psC.tile([TH, TILE], FP, tag="gate")
                nc.tensor.matmul(t1ps[:], ow1_sb[:, t * TH:(t + 1) * TH], combT[:],
                                 start=True, stop=True)
                th = tpool.tile([TH, TILE], BF, tag="th")
                nc.scalar.activation(th[:], t1ps[:], AF.Relu,
                                     bias=ob1_sb[:, t:t + 1])
                t2ps = psD.tile([1, TILE], FP, tag="small")
                nc.tensor.matmul(t2ps[:], ow2_sb[:, t:t + 1], th[:],
                                 start=True, stop=True)
                tnh = opool.tile([1, TILE], FP, tag=f"tanh{t}")
                nc.scalar.activation(tnh[:], t2ps[:], AF.Tanh,
                                     scale=0.5, bias=ob2h_sb[:, t:t + 1])
                # 0.5*tanh + 0.5 = sigmoid, interleaved into [s, T] order
                nc.vector.tensor_scalar(
                    orow[:, :, t], tnh[:], 0.5, 0.5,
                    op0=ALU.mult, op1=ALU.add)

            nc.sync.dma_start(
                out[:, i * TILE * T:(i + 1) * TILE * T],
                orow[:].rearrange("p a b -> p (a b)"))

    _split_excess_waits(nc)
    return nc


def _pack_inputs(x, sw1, sb1, sw2, sb2, dw1, db1, dw2, db2,
                 tw1, tb1, tw2, tb2, gw1, gb1, gw2, gb2,
                 ow1, ob1, ow2, ob2):
    f = np.float32
    w1_all = np.concatenate([sw1, dw1, tw1.reshape(T * 2, IN, H)], 0).astype(f)
    w2_all = np.concatenate([sw2, dw2, tw2.reshape(T * 2, H, D)], 0).astype(f)
    b1_all = np.concatenate([sb1, db1, tb1.reshape(T * 2, H)], 0).astype(f)
    b2_all = np.concatenate([sb2, db2, tb2.reshape(T * 2, D)], 0).astype(f)

    common = {
        "w1": np.ascontiguousarray(
            w1_all.reshape(NEXP, KC, 128, HC, 128).transpose(2, 0, 1, 3, 4)
            .reshape(128, NEXP * KC * HC * 128)).astype(NPBF),
        "w2": np.ascontiguousarray(
            w2_all.reshape(NEXP, HC, 128, 128).transpose(2, 0, 1, 3)
            .reshape(128, NEXP * HC * 128)).astype(NPBF),
        "b1": np.ascontiguousarray(
            b1_all.reshape(NEXP, HC, 128).transpose(2, 1, 0).reshape(128, HC * NEXP)),
        "b2": np.ascontiguousarray(
            np.tile(b2_all[:, None, :], (1, SC, 1)).reshape(1, NEXP * TILE)
        ).astype(NPBF),
        "gw1": np.ascontiguousarray(
            gw1.reshape(T, KC, 128, GH).transpose(2, 1, 0, 3)
            .reshape(128, KC * T * GH)).astype(NPBF),
        "gb1": np.ascontiguousarray(gb1.reshape(T * GH, 1).astype(f)),
        "gw2": np.ascontiguousarray(gw2.reshape(T * GH, NE)).astype(NPBF),
        "gb2": np.ascontiguousarray(gb2.reshape(1, T * NE)).astype(NPBF),
        "ow1": np.ascontiguousarray(
            ow1.transpose(1, 0, 2).reshape(D, T * TH)).astype(NPBF),
        "ob1": np.ascontiguousarray(ob1.T.astype(f)),
        "ow2": np.ascontiguousarray(
            ow2.transpose(1, 0, 2).reshape(TH, T)).astype(NPBF),
        "ob2h": np.ascontiguousarray((0.5 * ob2).reshape(1, T).astype(f)),
    }

    xTfull = np.ascontiguousarray(x.astype(f).T.astype(NPBF))   # [IN, B]
    in_maps = []
    for c in range(NCORES):
        m = dict(common)
        m["xT"] = np.ascontiguousarray(xTfull[:, c * BC:(c + 1) * BC])
        in_maps.append(m)
    return in_maps


def kernel(**inputs):
    inputs = {k: np.asarray(v) for k, v in inputs.items()}
    inputs.pop("domain_ids", None)   # unused by the reference computation
    x = inputs.pop("x")

    if "nc" not in _CACHE:
        _CACHE["nc"] = _build_bass()
    nc = _CACHE["nc"]

    in_maps = _pack_inputs(x=x, **inputs)

    trace = bool(int(os.environ.get("KERNEL_TRACE", "0")))
    res = run_bass_kernel_spmd(nc, in_maps, core_ids=list(range(NCORES)),
                               trace=trace)
    _CACHE["last_results"] = res

    out = np.concatenate(
        [res.results[c]["out"].reshape(BC, T) for c in range(NCORES)], axis=0)
    return out.astype(np.float32)


# revision 19
# speedup vs baseline: 3.5769x; 1.0237x over previous
"""Trainium2 Bass kernel for the MultiLayerPLEMD (moe_routing) problem.

Data-parallel over the batch axis: 16384 samples -> 8 NeuronCores x 2048.
All expert/gate/tower weights are replicated on every core. No collectives.

Network (per sample x[512]):
  14 expert MLPs (512 ->relu 256 ->relu 128): 4 shared + 6 domain + 2x2 task
  2 task gates: softmax(relu(x@gw1+gb1)@gw2+gb2) over 12 experts
                (10 common + 2 task-specific)
  comb[t] = sum_e g[t,e] * expert_e(x)          (per-sample weighted combine)
  out[t]  = sigmoid(relu(comb@ow1+ob1)@ow2+ob2)

On-chip strategy per core (2048 samples = 4 tiles of 512):
  - x is transposed on host to xT[512, B]; all L1 matmuls run feature-major
    (lhsT = W1 128x128 chunk, rhs = xT chunk, N=512 samples).
  - Expert L2 runs sample-major (lhsT = relu'd h chunk [128h,128s]) so expert
    outputs land as [samples, D] which makes the per-sample weighted combine a
    per-partition-scalar op on the vector engine (scalar_tensor_tensor).
  - Gates: logits computed sample-major [s, 12]; softmax along the free dim.
  - Towers: combined tile is PE-transposed back to feature-major; final
    sigmoid is computed as 0.5*tanh(0.5x + 0.5*b)+0.5 so every activation
    (relu/exp/copy/tanh) lives in the single `exp_and_others` ACT table set.
  - Matmul operands are bf16 (fp32 matmuls lower to TWO array passes on
    trn2); accumulation in PSUM and all softmax/bias/combine math is fp32.
"""

import os
import sys

for _p in ("/opt/trn_rl_repo",):
    if _p not in sys.path and os.path.isdir(_p):
        sys.path.insert(0, _p)

import numpy as np
import ml_dtypes
from contextlib import ExitStack

import concourse.bass as bass
import concourse.mybir as mybir
import concourse.tile as tile
from concourse.bass_utils import run_bass_kernel_spmd
from concourse.masks import make_identity

FP = mybir.dt.float32
BF = mybir.dt.bfloat16
NPBF = ml_dtypes.bfloat16
AF = mybir.ActivationFunctionType
ALU = mybir.AluOpType

NCORES = 8
B = 16384
BC = B // NCORES          # samples per core
IN, H, D = 512, 256, 128
NEXP = 14                 # 4 shared + 6 domain + 2*2 task experts
NE = 12                   # experts seen by each task gate
T = 2
GH = 64
TH = 64
TILE = 512                # samples per on-chip tile
NTILES = BC // TILE       # 4
SC = TILE // 128          # sample chunks per tile
KC = IN // 128            # contraction chunks for L1
HC = H // 128             # contraction chunks for L2

_CACHE = {}
# matmul lhsT/rhs at SBUF partition offset 64 compiles but faults at runtime
# on this toolchain, so gate L2 copies each task's operands to partition 0.
STACK_GATE_L2 = bool(int(os.environ.get("STACK_GATE_L2", "0")))


def _tasks_of_expert(e):
    """(task, gate_column) pairs that expert e feeds."""
    if e < 10:
        return [(0, e), (1, e)]
    if e < 12:
        return [(0, e)]          # task-0 experts -> gate cols 10, 11
    return [(1, e - 2)]          # task-1 experts -> gate cols 10, 11


def _split_excess_waits(nc, cap=1):
    """walrus's per-engine instruction structs carry a single sync-wait
    command; any scheduled instruction with >1 waits fails codegen ("Too many
    sync wait commands").  Move excess waits onto NoOp instructions inserted
    right before the offending instruction on the same engine."""
    for b in nc.m.functions[0].blocks:
        out = []
        changed = False
        for ins in b.instructions:
            si = getattr(ins, "sync_info", None)
            if si is not None and len(si.on_wait) > cap:
                extra = si.on_wait[:-cap]
                for j, w in enumerate(extra):
                    out.append(mybir.InstNoOp(
                        name=f"{ins.name}-wsplit{j}",
                        engine=ins.engine,
                        ins=[], outs=[],
                        sync_info=mybir.SyncInfo(on_wait=[w], on_update=[]),
                    ))
                ins.sync_info = mybir.SyncInfo(
                    on_wait=si.on_wait[-cap:], on_update=si.on_update)
                changed = True
            out.append(ins)
        if changed:
            b.instructions = out


def _build_bass():
    nc = bass.Bass(trn_type="TRN2", target_bir_lowering=False)

    xT = nc.dram_tensor("xT", [IN, BC], BF, kind="ExternalInput")
    w1 = nc.dram_tensor("w1", [128, NEXP * KC * HC * 128], BF, kind="ExternalInput")
    w2 = nc.dram_tensor("w2", [128, NEXP * HC * 128], BF, kind="ExternalInput")
    b1 = nc.dram_tensor("b1", [128, HC * NEXP], FP, kind="ExternalInput")
    b2 = nc.dram_tensor("b2", [1, NEXP * TILE], BF, kind="ExternalInput")
    gw1 = nc.dram_tensor("gw1", [128, KC * T * GH], BF, kind="ExternalInput")
    gb1 = nc.dram_tensor("gb1", [T * GH, 1], FP, kind="ExternalInput")
    gw2 = nc.dram_tensor("gw2", [T * GH, NE], BF, kind="ExternalInput")
    gb2 = nc.dram_tensor("gb2", [1, T * NE], BF, kind="ExternalInput")
    ow1 = nc.dram_tensor("ow1", [D, T * TH], BF, kind="ExternalInput")
    ob1 = nc.dram_tensor("ob1", [TH, T], FP, kind="ExternalInput")
    ow2 = nc.dram_tensor("ow2", [TH, T], BF, kind="ExternalInput")
    ob2h = nc.dram_tensor("ob2h", [1, T], FP, kind="ExternalInput")
    out = nc.dram_tensor("out", [1, BC * T], FP, kind="ExternalOutput")

    with tile.TileContext(nc) as tc, ExitStack() as ctx:
        wpool = ctx.enter_context(tc.tile_pool(name="weights", bufs=1))
        xpool = ctx.enter_context(tc.tile_pool(name="x", bufs=2))
        hpool = ctx.enter_context(tc.tile_pool(name="h", bufs=2))
        eopool = ctx.enter_context(tc.tile_pool(name="eo", bufs=3))
        combpool = ctx.enter_context(tc.tile_pool(name="comb", bufs=2))
        gpool = ctx.enter_context(tc.tile_pool(name="g", bufs=2))
        tpool = ctx.enter_context(tc.tile_pool(name="tower", bufs=2))
        opool = ctx.enter_context(tc.tile_pool(name="outrow", bufs=2))
        psA = ctx.enter_context(tc.tile_pool(name="psA", bufs=3, space=bass.MemorySpace.PSUM))
        psB = ctx.enter_context(tc.tile_pool(name="psB", bufs=2, space=bass.MemorySpace.PSUM))
        psC = ctx.enter_context(tc.tile_pool(name="psC", bufs=2, space=bass.MemorySpace.PSUM))
        psD = ctx.enter_context(tc.tile_pool(name="psD", bufs=1, space=bass.MemorySpace.PSUM))

        # ---- resident weights -------------------------------------------
        # Small gate/tower weights + the first x tile first, so compute can
        # start while the bulk expert weights stream in per-expert.
        gw1_sb = wpool.tile([128, KC * T * GH], BF)
        nc.sync.dma_start(gw1_sb[:], gw1[:])
        gb1_sb = wpool.tile([T * GH, 1], FP)
        nc.sync.dma_start(gb1_sb[:], gb1[:])
        gw2_sb = wpool.tile([T * GH, NE], BF)
        nc.sync.dma_start(gw2_sb[:], gw2[:])
        gb2_sb = wpool.tile([1, T * NE], BF)
        nc.sync.dma_start(gb2_sb[:], gb2[:])
        b1_sb = wpool.tile([128, HC * NEXP], FP)
        nc.sync.dma_start(b1_sb[:], b1[:])
        b2_sb = wpool.tile([1, NEXP * TILE], BF)
        nc.sync.dma_start(b2_sb[:], b2[:])

        w1_sb = wpool.tile([128, NEXP * KC * HC * 128], BF)
        w2_sb = wpool.tile([128, NEXP * HC * 128], BF)
        for e in range(NEXP):
            o = e * KC * HC * 128
            nc.sync.dma_start(w1_sb[:, o:o + KC * HC * 128],
                              w1[:, o:o + KC * HC * 128])
            o2 = e * HC * 128
            nc.sync.dma_start(w2_sb[:, o2:o2 + HC * 128],
                              w2[:, o2:o2 + HC * 128])

        ow1_sb = wpool.tile([D, T * TH], BF)
        nc.sync.dma_start(ow1_sb[:], ow1[:])
        ob1_sb = wpool.tile([TH, T], FP)
        nc.sync.dma_start(ob1_sb[:], ob1[:])
        ow2_sb = wpool.tile([TH, T], BF)
        nc.sync.dma_start(ow2_sb[:], ow2[:])
        ob2h_sb = wpool.tile([1, T], FP)
        nc.sync.dma_start(ob2h_sb[:], ob2h[:])

        ident = wpool.tile([128, 128], FP)
        make_identity(nc, ident[:])
        ones_sb = wpool.tile([1, 128], BF)
        nc.vector.memset(ones_sb[:], 1.0)

        def w1_sl(e, kc, hc):
            o = ((e * KC + kc) * HC + hc) * 128
            return w1_sb[:, o:o + 128]

        def w2_sl(e, hc):
            o = (e * HC + hc) * 128
            return w2_sb[:, o:o + 128]

        # ---- per-tile pipeline ------------------------------------------
        for i in range(NTILES):
            xt = []
            for kc in range(KC):
                t_ = xpool.tile([128, TILE], BF, tag=f"xt{kc}")
                nc.sync.dma_start(
                    t_[:], xT[kc * 128:(kc + 1) * 128, i * TILE:(i + 1) * TILE])
                xt.append(t_)

            # ---------------- gates ----------------
            # both tasks' gate hiddens stacked on the partition axis (2*64)
            gps = psC.tile([T * GH, TILE], FP, tag="gate")
            for kc in range(KC):
                nc.tensor.matmul(gps[:], gw1_sb[:, kc * 128:(kc + 1) * 128],
                                 xt[kc][:],
                                 start=(kc == 0), stop=(kc == KC - 1))
            if STACK_GATE_L2:
                g1 = gpool.tile([T * GH, TILE], BF, tag="g1")
                nc.scalar.activation(g1[:], gps[:], AF.Relu, bias=gb1_sb[:])
                g1s = [g1[t * GH:(t + 1) * GH, :] for t in range(T)]
                gw2s = [gw2_sb[t * GH:(t + 1) * GH, :] for t in range(T)]
            else:
                g1s, gw2s = [], []
                for t in range(T):
                    g1t = gpool.tile([GH, TILE], BF, tag=f"g1_{t}", name=f"g1_{t}")
                    nc.scalar.activation(g1t[:], gps[t * GH:(t + 1) * GH, :],
                                         AF.Relu, bias=gb1_sb[t * GH:(t + 1) * GH, :])
                    g1s.append(g1t[:, :])
                    gw2t = gpool.tile([GH, NE], BF, tag=f"gw2_{t}", name=f"gw2_{t}")
                    nc.vector.tensor_copy(gw2t[:], gw2_sb[t * GH:(t + 1) * GH, :])
                    gw2s.append(gw2t[:, :])

            lps = psD.tile([128, SC * T * NE], FP, tag="small")
            for sc in range(SC):
                o = sc * (T * NE)
                nc.tensor.matmul(lps[:, o:o + T * NE], ones_sb[:], gb2_sb[:],
                                 start=True, stop=False, skip_group_check=True)
                for t in range(T):
                    nc.tensor.matmul(lps[:, o + t * NE:o + (t + 1) * NE],
                                     g1s[t][:, sc * 128:(sc + 1) * 128],
                                     gw2s[t],
                                     start=False, stop=(t == T - 1),
                                     skip_group_check=True)

            exp_sb = gpool.tile([128, SC * T * NE], FP, tag="exp")
            nc.scalar.activation(exp_sb[:], lps[:], AF.Exp)
            sums = gpool.tile([128, SC * T], FP, tag="gsum")
            nc.vector.tensor_reduce(
                sums[:],
                exp_sb[:].rearrange("p (g e) -> p g e", e=NE),
                axis=mybir.AxisListType.X, op=ALU.add)
            rec = gpool.tile([128, SC * T], FP, tag="grec")
            nc.vector.reciprocal(rec[:], sums[:])
            g_sb = gpool.tile([128, SC * T * NE], FP, tag="g")
            for idx in range(SC * T):
                nc.vector.tensor_scalar_mul(
                    g_sb[:, idx * NE:(idx + 1) * NE],
                    exp_sb[:, idx * NE:(idx + 1) * NE],
                    rec[:, idx:idx + 1])

            # ---------------- experts + combine ----------------
            comb = [combpool.tile([128, TILE], FP, tag=f"comb{t}", name=f"comb{t}")
                    for t in range(T)]
            comb_started = [False] * T

            for e in range(NEXP):
                h_tiles = []
                for hc in range(HC):
                    l1ps = psA.tile([128, TILE], FP, tag="l1")
                    for kc in range(KC):
                        nc.tensor.matmul(l1ps[:], w1_sl(e, kc, hc), xt[kc][:],
                                         start=(kc == 0), stop=(kc == KC - 1))
                    ht = hpool.tile([128, TILE], BF, tag=f"h{hc}")
                    nc.scalar.activation(ht[:], l1ps[:], AF.Relu,
                                         bias=b1_sb[:, hc * NEXP + e:hc * NEXP + e + 1])
                    h_tiles.append(ht)

                l2ps = psB.tile([128, TILE], FP, tag="l2")
                nc.tensor.matmul(l2ps[:], ones_sb[:],
                                 b2_sb[:, e * TILE:(e + 1) * TILE],
                                 start=True, stop=False, skip_group_check=True)
                for sc in range(SC):
                    sl = l2ps[:, sc * 128:(sc + 1) * 128]
                    for hc in range(HC):
                        nc.tensor.matmul(sl, h_tiles[hc][:, sc * 128:(sc + 1) * 128],
                                         w2_sl(e, hc),
                                         start=False,
                                         stop=(sc == SC - 1 and hc == HC - 1),
                                         skip_group_check=True)
                eo = eopool.tile([128, TILE], BF, tag="eo")
                nc.scalar.activation(eo[:], l2ps[:], AF.Relu)

                for (t, gcol) in _tasks_of_expert(e):
                    eng = nc.vector
                    for sc in range(SC):
                        c_sl = comb[t][:, sc * 128:(sc + 1) * 128]
                        e_sl = eo[:, sc * 128:(sc + 1) * 128]
                        g_ap = g_sb[:, sc * (T * NE) + t * NE + gcol:
                                    sc * (T * NE) + t * NE + gcol + 1]
                        if not comb_started[t]:
                            eng.tensor_scalar_mul(c_sl, e_sl, g_ap)
                        else:
                            eng.scalar_tensor_tensor(
                                c_sl, e_sl, g_ap, c_sl, op0=ALU.mult, op1=ALU.add)
                    comb_started[t] = True

            # ---------------- towers ----------------
            orow = opool.tile([1, TILE, T], FP, tag="orow")
            for t in range(T):
                trps = psB.tile([128, TILE], FP, tag="l2")
                for sc in range(SC):
                    nc.tensor.transpose(trps[:, sc * 128:(sc + 1) * 128],
                                        comb[t][:, sc * 128:(sc + 1) * 128],
                                        ident[:])
                combT = tpool.tile([128, TILE], BF, tag="combT")
                nc.scalar.copy(combT[:], trps[:])
                t1ps = psC.tile([TH, TILE], FP, tag="gate")
                nc.tensor.matmul(t1ps[:], ow1_sb[:, t * TH:(t + 1) * TH], combT[:],
                                 start=True, stop=True)
                th = tpool.tile([TH, TILE], BF, tag="th")
                nc.scalar.activation(th[:], t1ps[:], AF.Relu,
                                     bias=ob1_sb[:, t:t + 1])
                t2ps = psD.tile([1, TILE], FP, tag="small")
                nc.tensor.matmul(t2ps[:], ow2_sb[:, t:t + 1], th[:],
                                 start=True, stop=True)
                tnh = opool.tile([1, TILE], FP, tag=f"tanh{t}")
                nc.scalar.activation(tnh[:], t2ps[:], AF.Tanh,
                                     scale=0.5, bias=ob2h_sb[:, t:t + 1])
                # 0.5*tanh + 0.5 = sigmoid, interleaved into [s, T] order
                nc.vector.tensor_scalar(
                    orow[:, :, t], tnh[:], 0.5, 0.5,
                    op0=ALU.mult, op1=ALU.add)

            nc.sync.dma_start(
                out[:, i * TILE * T:(i + 1) * TILE * T],
                orow[:].rearrange("p a b -> p (a b)"))

    _split_excess_waits(nc)
    return nc


def _pack_inputs(x, sw1, sb1, sw2, sb2, dw1, db1, dw2, db2,
                 tw1, tb1, tw2, tb2, gw1, gb1, gw2, gb2,
                 ow1, ob1, ow2, ob2):
    f = np.float32
    w1_all = np.concatenate([sw1, dw1, tw1.reshape(T * 2, IN, H)], 0).astype(f)
    w2_all = np.concatenate([sw2, dw2, tw2.reshape(T * 2, H, D)], 0).astype(f)
    b1_all = np.concatenate([sb1, db1, tb1.reshape(T * 2, H)], 0).astype(f)
    b2_all = np.concatenate([sb2, db2, tb2.reshape(T * 2, D)], 0).astype(f)

    common = {
        "w1": np.ascontiguousarray(
            w1_all.reshape(NEXP, KC, 128, HC, 128).transpose(2, 0, 1, 3, 4)
            .reshape(128, NEXP * KC * HC * 128)).astype(NPBF),
        "w2": np.ascontiguousarray(
            w2_all.reshape(NEXP, HC, 128, 128).transpose(2, 0, 1, 3)
            .reshape(128, NEXP * HC * 128)).astype(NPBF),
        "b1": np.ascontiguousarray(
            b1_all.reshape(NEXP, HC, 128).transpose(2, 1, 0).reshape(128, HC * NEXP)),
        "b2": np.ascontiguousarray(
            np.tile(b2_all[:, None, :], (1, SC, 1)).reshape(1, NEXP * TILE)
        ).astype(NPBF),
        "gw1": np.ascontiguousarray(
            gw1.reshape(T, KC, 128, GH).transpose(2, 1, 0, 3)
            .reshape(128, KC * T * GH)).astype(NPBF),
        "gb1": np.ascontiguousarray(gb1.reshape(T * GH, 1).astype(f)),
        "gw2": np.ascontiguousarray(gw2.reshape(T * GH, NE)).astype(NPBF),
        "gb2": np.ascontiguousarray(gb2.reshape(1, T * NE)).astype(NPBF),
        "ow1": np.ascontiguousarray(
            ow1.transpose(1, 0, 2).reshape(D, T * TH)).astype(NPBF),
        "ob1": np.ascontiguousarray(ob1.T.astype(f)),
        "ow2": np.ascontiguousarray(
            ow2.transpose(1, 0, 2).reshape(TH, T)).astype(NPBF),
        "ob2h": np.ascontiguousarray((0.5 * ob2).reshape(1, T).astype(f)),
    }

    xTfull = np.ascontiguousarray(x.astype(f).T.astype(NPBF))   # [IN, B]
    in_maps = []
    for c in range(NCORES):
        m = dict(common)
        m["xT"] = np.ascontiguousarray(xTfull[:, c * BC:(c + 1) * BC])
        in_maps.append(m)
    return in_maps


def kernel(**inputs):
    inputs = {k: np.asarray(v) for k, v in inputs.items()}
    inputs.pop("domain_ids", None)   # unused by the reference computation
    x = inputs.pop("x")

    if "nc" not in _CACHE:
        _CACHE["nc"] = _build_bass()
    nc = _CACHE["nc"]

    in_maps = _pack_inputs(x=x, **inputs)

    trace = bool(int(os.environ.get("KERNEL_TRACE", "0")))
    res = run_bass_kernel_spmd(nc, in_maps, core_ids=list(range(NCORES)),
                               trace=trace)
    _CACHE["last_results"] = res

    out = np.concatenate(
        [res.results[c]["out"].reshape(BC, T) for c in range(NCORES)], axis=0)
    return out.astype(np.float32)


# revision 20
# speedup vs baseline: 3.9446x; 1.1028x over previous
"""Trainium2 Bass kernel for the MultiLayerPLEMD (moe_routing) problem.

Data-parallel over the batch axis: 16384 samples -> 8 NeuronCores x 2048.
All expert/gate/tower weights are replicated on every core. No collectives.

Network (per sample x[512]):
  14 expert MLPs (512 ->relu 256 ->relu 128): 4 shared + 6 domain + 2x2 task
  2 task gates: softmax(relu(x@gw1+gb1)@gw2+gb2) over 12 experts
                (10 common + 2 task-specific)
  comb[t] = sum_e g[t,e] * expert_e(x)          (per-sample weighted combine)
  out[t]  = sigmoid(relu(comb@ow1+ob1)@ow2+ob2)

On-chip strategy per core (2048 samples = 4 tiles of 512):
  - x is transposed on host to xT[512, B]; all L1 matmuls run feature-major
    (lhsT = W1 128x128 chunk, rhs = xT chunk, N=512 samples).
  - Expert L2 runs sample-major (lhsT = relu'd h chunk [128h,128s]) so expert
    outputs land as [samples, D] which makes the per-sample weighted combine a
    per-partition-scalar op on the vector engine (scalar_tensor_tensor).
  - Gates: logits computed sample-major [s, 12]; softmax along the free dim.
  - Towers: combined tile is PE-transposed back to feature-major; final
    sigmoid is computed as 0.5*tanh(0.5x + 0.5*b)+0.5 so every activation
    (relu/exp/copy/tanh) lives in the single `exp_and_others` ACT table set.
  - Matmul operands are bf16 (fp32 matmuls lower to TWO array passes on
    trn2); accumulation in PSUM and all softmax/bias/combine math is fp32.
"""

import os
import sys

for _p in ("/opt/trn_rl_repo",):
    if _p not in sys.path and os.path.isdir(_p):
        sys.path.insert(0, _p)

import numpy as np
import ml_dtypes
from contextlib import ExitStack

import concourse.bass as bass
import concourse.mybir as mybir
import concourse.tile as tile
from concourse.bass_utils import run_bass_kernel_spmd
from concourse.masks import make_identity

FP = mybir.dt.float32
BF = mybir.dt.bfloat16
NPBF = ml_dtypes.bfloat16
AF = mybir.ActivationFunctionType
ALU = mybir.AluOpType

NCORES = 8
B = 16384
BC = B // NCORES          # samples per core
IN, H, D = 512, 256, 128
NEXP = 14                 # 4 shared + 6 domain + 2*2 task experts
NE = 12                   # experts seen by each task gate
T = 2
GH = 64
TH = 64
TILE = 512                # samples per on-chip tile
NTILES = BC // TILE       # 4
SC = TILE // 128          # sample chunks per tile
KC = IN // 128            # contraction chunks for L1
HC = H // 128             # contraction chunks for L2

_CACHE = {}
# matmul lhsT/rhs at SBUF partition offset 64 compiles but faults at runtime
# on this toolchain, so gate L2 copies each task's operands to partition 0.
STACK_GATE_L2 = bool(int(os.environ.get("STACK_GATE_L2", "0")))


def _tasks_of_expert(e):
    """(task, gate_column) pairs that expert e feeds."""
    if e < 10:
        return [(0, e), (1, e)]
    if e < 12:
        return [(0, e)]          # task-0 experts -> gate cols 10, 11
    return [(1, e - 2)]          # task-1 experts -> gate cols 10, 11


def _split_excess_waits(nc, cap=1):
    """walrus's per-engine instruction structs carry a single sync-wait
    command; any scheduled instruction with >1 waits fails codegen ("Too many
    sync wait commands").  Move excess waits onto NoOp instructions inserted
    right before the offending instruction on the same engine."""
    for b in nc.m.functions[0].blocks:
        out = []
        changed = False
        for ins in b.instructions:
            si = getattr(ins, "sync_info", None)
            if si is not None and len(si.on_wait) > cap:
                extra = si.on_wait[:-cap]
                for j, w in enumerate(extra):
                    out.append(mybir.InstNoOp(
                        name=f"{ins.name}-wsplit{j}",
                        engine=ins.engine,
                        ins=[], outs=[],
                        sync_info=mybir.SyncInfo(on_wait=[w], on_update=[]),
                    ))
                ins.sync_info = mybir.SyncInfo(
                    on_wait=si.on_wait[-cap:], on_update=si.on_update)
                changed = True
            out.append(ins)
        if changed:
            b.instructions = out


def _build_bass():
    nc = bass.Bass(trn_type="TRN2", target_bir_lowering=False)

    xT = nc.dram_tensor("xT", [IN, BC], BF, kind="ExternalInput")
    w1 = nc.dram_tensor("w1", [128, NEXP * KC * HC * 128], BF, kind="ExternalInput")
    w2 = nc.dram_tensor("w2", [128, NEXP * HC * 128], BF, kind="ExternalInput")
    b1 = nc.dram_tensor("b1", [128, HC * NEXP], FP, kind="ExternalInput")
    b2 = nc.dram_tensor("b2", [1, NEXP * TILE], BF, kind="ExternalInput")
    gw1 = nc.dram_tensor("gw1", [128, KC * T * GH], BF, kind="ExternalInput")
    gb1 = nc.dram_tensor("gb1", [T * GH, 1], FP, kind="ExternalInput")
    gw2 = nc.dram_tensor("gw2", [T * GH, NE], BF, kind="ExternalInput")
    gb2 = nc.dram_tensor("gb2", [1, T * NE], BF, kind="ExternalInput")
    ow1 = nc.dram_tensor("ow1", [D, T * TH], BF, kind="ExternalInput")
    ob1 = nc.dram_tensor("ob1", [TH, T], FP, kind="ExternalInput")
    ow2 = nc.dram_tensor("ow2", [TH, T], BF, kind="ExternalInput")
    ob2h = nc.dram_tensor("ob2h", [1, T], FP, kind="ExternalInput")
    out = nc.dram_tensor("out", [1, BC * T], FP, kind="ExternalOutput")

    with tile.TileContext(nc) as tc, ExitStack() as ctx:
        wpool = ctx.enter_context(tc.tile_pool(name="weights", bufs=1))
        xpool = ctx.enter_context(tc.tile_pool(name="x", bufs=2))
        hpool = ctx.enter_context(tc.tile_pool(name="h", bufs=2))
        eopool = ctx.enter_context(tc.tile_pool(name="eo", bufs=3))
        combpool = ctx.enter_context(tc.tile_pool(name="comb", bufs=2))
        gpool = ctx.enter_context(tc.tile_pool(name="g", bufs=2))
        tpool = ctx.enter_context(tc.tile_pool(name="tower", bufs=2))
        opool = ctx.enter_context(tc.tile_pool(name="outrow", bufs=2))
        psA = ctx.enter_context(tc.tile_pool(name="psA", bufs=3, space=bass.MemorySpace.PSUM))
        psB = ctx.enter_context(tc.tile_pool(name="psB", bufs=2, space=bass.MemorySpace.PSUM))
        psC = ctx.enter_context(tc.tile_pool(name="psC", bufs=2, space=bass.MemorySpace.PSUM))
        psD = ctx.enter_context(tc.tile_pool(name="psD", bufs=1, space=bass.MemorySpace.PSUM))

        # ---- resident weights -------------------------------------------
        # Small gate/tower weights + the first x tile first, so compute can
        # start while the bulk expert weights stream in per-expert.
        gw1_sb = wpool.tile([128, KC * T * GH], BF)
        nc.sync.dma_start(gw1_sb[:], gw1[:])
        gb1_sb = wpool.tile([T * GH, 1], FP)
        nc.sync.dma_start(gb1_sb[:], gb1[:])
        gw2_sb = wpool.tile([T * GH, NE], BF)
        nc.sync.dma_start(gw2_sb[:], gw2[:])
        gb2_sb = wpool.tile([1, T * NE], BF)
        nc.sync.dma_start(gb2_sb[:], gb2[:])
        b1_sb = wpool.tile([128, HC * NEXP], FP)
        nc.sync.dma_start(b1_sb[:], b1[:])
        b2_sb = wpool.tile([1, NEXP * TILE], BF)
        nc.sync.dma_start(b2_sb[:], b2[:])

        w1_sb = wpool.tile([128, NEXP * KC * HC * 128], BF)
        w2_sb = wpool.tile([128, NEXP * HC * 128], BF)
        for e in range(NEXP):
            o = e * KC * HC * 128
            nc.sync.dma_start(w1_sb[:, o:o + KC * HC * 128],
                              w1[:, o:o + KC * HC * 128])
            o2 = e * HC * 128
            nc.sync.dma_start(w2_sb[:, o2:o2 + HC * 128],
                              w2[:, o2:o2 + HC * 128])

        ow1_sb = wpool.tile([D, T * TH], BF)
        nc.sync.dma_start(ow1_sb[:], ow1[:])
        ob1_sb = wpool.tile([TH, T], FP)
        nc.sync.dma_start(ob1_sb[:], ob1[:])
        ow2_sb = wpool.tile([TH, T], BF)
        nc.sync.dma_start(ow2_sb[:], ow2[:])
        ob2h_sb = wpool.tile([1, T], FP)
        nc.sync.dma_start(ob2h_sb[:], ob2h[:])

        ident = wpool.tile([128, 128], FP)
        make_identity(nc, ident[:])
        ones_sb = wpool.tile([1, 128], BF)
        nc.vector.memset(ones_sb[:], 1.0)

        def w1_sl(e, kc, hc):
            o = ((e * KC + kc) * HC + hc) * 128
            return w1_sb[:, o:o + 128]

        def w2_sl(e, hc):
            o = (e * HC + hc) * 128
            return w2_sb[:, o:o + 128]

        # ---- per-tile pipeline ------------------------------------------
        for i in range(NTILES):
            xt = []
            for kc in range(KC):
                t_ = xpool.tile([128, TILE], BF, tag=f"xt{kc}")
                # gpsimd SWDGE ring: x tiles must not queue behind the bulk
                # weight stream on the sync HWDGE ring
                nc.gpsimd.dma_start(
                    t_[:], xT[kc * 128:(kc + 1) * 128, i * TILE:(i + 1) * TILE])
                xt.append(t_)

            # ---------------- gates ----------------
            # both tasks' gate hiddens stacked on the partition axis (2*64)
            gps = psC.tile([T * GH, TILE], FP, tag="gate")
            for kc in range(KC):
                nc.tensor.matmul(gps[:], gw1_sb[:, kc * 128:(kc + 1) * 128],
                                 xt[kc][:],
                                 start=(kc == 0), stop=(kc == KC - 1))
            if STACK_GATE_L2:
                g1 = gpool.tile([T * GH, TILE], BF, tag="g1")
                nc.scalar.activation(g1[:], gps[:], AF.Relu, bias=gb1_sb[:])
                g1s = [g1[t * GH:(t + 1) * GH, :] for t in range(T)]
                gw2s = [gw2_sb[t * GH:(t + 1) * GH, :] for t in range(T)]
            else:
                g1s, gw2s = [], []
                for t in range(T):
                    g1t = gpool.tile([GH, TILE], BF, tag=f"g1_{t}", name=f"g1_{t}")
                    nc.scalar.activation(g1t[:], gps[t * GH:(t + 1) * GH, :],
                                         AF.Relu, bias=gb1_sb[t * GH:(t + 1) * GH, :])
                    g1s.append(g1t[:, :])
                    gw2t = gpool.tile([GH, NE], BF, tag=f"gw2_{t}", name=f"gw2_{t}")
                    nc.vector.tensor_copy(gw2t[:], gw2_sb[t * GH:(t + 1) * GH, :])
                    gw2s.append(gw2t[:, :])

            lps = psD.tile([128, SC * T * NE], FP, tag="small")
            for sc in range(SC):
                o = sc * (T * NE)
                nc.tensor.matmul(lps[:, o:o + T * NE], ones_sb[:], gb2_sb[:],
                                 start=True, stop=False, skip_group_check=True)
                for t in range(T):
                    nc.tensor.matmul(lps[:, o + t * NE:o + (t + 1) * NE],
                                     g1s[t][:, sc * 128:(sc + 1) * 128],
                                     gw2s[t],
                                     start=False, stop=(t == T - 1),
                                     skip_group_check=True)

            exp_sb = gpool.tile([128, SC * T * NE], FP, tag="exp")
            nc.scalar.activation(exp_sb[:], lps[:], AF.Exp)
            sums = gpool.tile([128, SC * T], FP, tag="gsum")
            nc.vector.tensor_reduce(
                sums[:],
                exp_sb[:].rearrange("p (g e) -> p g e", e=NE),
                axis=mybir.AxisListType.X, op=ALU.add)
            rec = gpool.tile([128, SC * T], FP, tag="grec")
            nc.vector.reciprocal(rec[:], sums[:])
            g_sb = gpool.tile([128, SC * T * NE], FP, tag="g")
            for idx in range(SC * T):
                nc.vector.tensor_scalar_mul(
                    g_sb[:, idx * NE:(idx + 1) * NE],
                    exp_sb[:, idx * NE:(idx + 1) * NE],
                    rec[:, idx:idx + 1])

            # ---------------- experts + combine ----------------
            comb = [combpool.tile([128, TILE], FP, tag=f"comb{t}", name=f"comb{t}")
                    for t in range(T)]
            comb_started = [False] * T

            for e in range(NEXP):
                h_tiles = []
                for hc in range(HC):
                    l1ps = psA.tile([128, TILE], FP, tag="l1")
                    for kc in range(KC):
                        nc.tensor.matmul(l1ps[:], w1_sl(e, kc, hc), xt[kc][:],
                                         start=(kc == 0), stop=(kc == KC - 1))
                    ht = hpool.tile([128, TILE], BF, tag=f"h{hc}")
                    nc.scalar.activation(ht[:], l1ps[:], AF.Relu,
                                         bias=b1_sb[:, hc * NEXP + e:hc * NEXP + e + 1])
                    h_tiles.append(ht)

                l2ps = psB.tile([128, TILE], FP, tag="l2")
                nc.tensor.matmul(l2ps[:], ones_sb[:],
                                 b2_sb[:, e * TILE:(e + 1) * TILE],
                                 start=True, stop=False, skip_group_check=True)
                for sc in range(SC):
                    sl = l2ps[:, sc * 128:(sc + 1) * 128]
                    for hc in range(HC):
                        nc.tensor.matmul(sl, h_tiles[hc][:, sc * 128:(sc + 1) * 128],
                                         w2_sl(e, hc),
                                         start=False,
                                         stop=(sc == SC - 1 and hc == HC - 1),
                                         skip_group_check=True)
                eo = eopool.tile([128, TILE], BF, tag="eo")
                nc.scalar.activation(eo[:], l2ps[:], AF.Relu)

                for (t, gcol) in _tasks_of_expert(e):
                    eng = nc.vector
                    for sc in range(SC):
                        c_sl = comb[t][:, sc * 128:(sc + 1) * 128]
                        e_sl = eo[:, sc * 128:(sc + 1) * 128]
                        g_ap = g_sb[:, sc * (T * NE) + t * NE + gcol:
                                    sc * (T * NE) + t * NE + gcol + 1]
                        if not comb_started[t]:
                            eng.tensor_scalar_mul(c_sl, e_sl, g_ap)
                        else:
                            eng.scalar_tensor_tensor(
                                c_sl, e_sl, g_ap, c_sl, op0=ALU.mult, op1=ALU.add)
                    comb_started[t] = True

            # ---------------- towers ----------------
            orow = opool.tile([1, TILE, T], FP, tag="orow")
            for t in range(T):
                trps = psB.tile([128, TILE], FP, tag="l2")
                for sc in range(SC):
                    nc.tensor.transpose(trps[:, sc * 128:(sc + 1) * 128],
                                        comb[t][:, sc * 128:(sc + 1) * 128],
                                        ident[:])
                combT = tpool.tile([128, TILE], BF, tag="combT")
                nc.scalar.copy(combT[:], trps[:])
                t1ps = psC.tile([TH, TILE], FP, tag="gate")
                nc.tensor.matmul(t1ps[:], ow1_sb[:, t * TH:(t + 1) * TH], combT[:],
                                 start=True, stop=True)
                th = tpool.tile([TH, TILE], BF, tag="th")
                nc.scalar.activation(th[:], t1ps[:], AF.Relu,
                                     bias=ob1_sb[:, t:t + 1])
                t2ps = psD.tile([1, TILE], FP, tag="small")
                nc.tensor.matmul(t2ps[:], ow2_sb[:, t:t + 1], th[:],
                                 start=True, stop=True)
                tnh = opool.tile([1, TILE], FP, tag=f"tanh{t}")
                nc.scalar.activation(tnh[:], t2ps[:], AF.Tanh,
                                     scale=0.5, bias=ob2h_sb[:, t:t + 1])
                # 0.5*tanh + 0.5 = sigmoid, interleaved into [s, T] order
                nc.vector.tensor_scalar(
                    orow[:, :, t], tnh[:], 0.5, 0.5,
                    op0=ALU.mult, op1=ALU.add)

            nc.sync.dma_start(
                out[:, i * TILE * T:(i + 1) * TILE * T],
                orow[:].rearrange("p a b -> p (a b)"))

    _split_excess_waits(nc)
    return nc


def _pack_inputs(x, sw1, sb1, sw2, sb2, dw1, db1, dw2, db2,
                 tw1, tb1, tw2, tb2, gw1, gb1, gw2, gb2,
                 ow1, ob1, ow2, ob2):
    f = np.float32
    w1_all = np.concatenate([sw1, dw1, tw1.reshape(T * 2, IN, H)], 0).astype(f)
    w2_all = np.concatenate([sw2, dw2, tw2.reshape(T * 2, H, D)], 0).astype(f)
    b1_all = np.concatenate([sb1, db1, tb1.reshape(T * 2, H)], 0).astype(f)
    b2_all = np.concatenate([sb2, db2, tb2.reshape(T * 2, D)], 0).astype(f)

    common = {
        "w1": np.ascontiguousarray(
            w1_all.reshape(NEXP, KC, 128, HC, 128).transpose(2, 0, 1, 3, 4)
            .reshape(128, NEXP * KC * HC * 128)).astype(NPBF),
        "w2": np.ascontiguousarray(
            w2_all.reshape(NEXP, HC, 128, 128).transpose(2, 0, 1, 3)
            .reshape(128, NEXP * HC * 128)).astype(NPBF),
        "b1": np.ascontiguousarray(
            b1_all.reshape(NEXP, HC, 128).transpose(2, 1, 0).reshape(128, HC * NEXP)),
        "b2": np.ascontiguousarray(
            np.tile(b2_all[:, None, :], (1, SC, 1)).reshape(1, NEXP * TILE)
        ).astype(NPBF),
        "gw1": np.ascontiguousarray(
            gw1.reshape(T, KC, 128, GH).transpose(2, 1, 0, 3)
            .reshape(128, KC * T * GH)).astype(NPBF),
        "gb1": np.ascontiguousarray(gb1.reshape(T * GH, 1).astype(f)),
        "gw2": np.ascontiguousarray(gw2.reshape(T * GH, NE)).astype(NPBF),
        "gb2": np.ascontiguousarray(gb2.reshape(1, T * NE)).astype(NPBF),
        "ow1": np.ascontiguousarray(
            ow1.transpose(1, 0, 2).reshape(D, T * TH)).astype(NPBF),
        "ob1": np.ascontiguousarray(ob1.T.astype(f)),
        "ow2": np.ascontiguousarray(
            ow2.transpose(1, 0, 2).reshape(TH, T)).astype(NPBF),
        "ob2h": np.ascontiguousarray((0.5 * ob2).reshape(1, T).astype(f)),
    }

    xTfull = np.ascontiguousarray(x.astype(f).T.astype(NPBF))   # [IN, B]
    in_maps = []
    for c in range(NCORES):
        m = dict(common)
        m["xT"] = np.ascontiguousarray(xTfull[:, c * BC:(c + 1) * BC])
        in_maps.append(m)
    return in_maps


def kernel(**inputs):
    inputs = {k: np.asarray(v) for k, v in inputs.items()}
    inputs.pop("domain_ids", None)   # unused by the reference computation
    x = inputs.pop("x")

    if "nc" not in _CACHE:
        _CACHE["nc"] = _build_bass()
    nc = _CACHE["nc"]

    in_maps = _pack_inputs(x=x, **inputs)

    trace = bool(int(os.environ.get("KERNEL_TRACE", "0")))
    res = run_bass_kernel_spmd(nc, in_maps, core_ids=list(range(NCORES)),
                               trace=trace)
    _CACHE["last_results"] = res

    out = np.concatenate(
        [res.results[c]["out"].reshape(BC, T) for c in range(NCORES)], axis=0)
    return out.astype(np.float32)


# revision 21
# speedup vs baseline: 4.5401x; 1.1510x over previous
"""Trainium2 Bass kernel for the MultiLayerPLEMD (moe_routing) problem.

Data-parallel over the batch axis: 16384 samples -> 8 NeuronCores x 2048.
All expert/gate/tower weights are replicated on every core. No collectives.

Network (per sample x[512]):
  14 expert MLPs (512 ->relu 256 ->relu 128): 4 shared + 6 domain + 2x2 task
  2 task gates: softmax(relu(x@gw1+gb1)@gw2+gb2) over 12 experts
                (10 common + 2 task-specific)
  comb[t] = sum_e g[t,e] * expert_e(x)          (per-sample weighted combine)
  out[t]  = sigmoid(relu(comb@ow1+ob1)@ow2+ob2)

On-chip strategy per core (2048 samples = 4 tiles of 512):
  - x is transposed on host to xT[512, B]; all L1 matmuls run feature-major
    (lhsT = W1 128x128 chunk, rhs = xT chunk, N=512 samples).
  - Expert L2 runs sample-major (lhsT = relu'd h chunk [128h,128s]) so expert
    outputs land as [samples, D] which makes the per-sample weighted combine a
    per-partition-scalar op on the vector engine (scalar_tensor_tensor).
  - Gates: logits computed sample-major [s, 12]; softmax along the free dim.
  - Towers: combined tile is PE-transposed back to feature-major; final
    sigmoid is computed as 0.5*tanh(0.5x + 0.5*b)+0.5 so every activation
    (relu/exp/copy/tanh) lives in the single `exp_and_others` ACT table set.
  - Matmul operands are bf16 (fp32 matmuls lower to TWO array passes on
    trn2); accumulation in PSUM and all softmax/bias/combine math is fp32.
"""

import os
import sys

for _p in ("/opt/trn_rl_repo",):
    if _p not in sys.path and os.path.isdir(_p):
        sys.path.insert(0, _p)

import numpy as np
import ml_dtypes
from contextlib import ExitStack

import concourse.bass as bass
import concourse.mybir as mybir
import concourse.tile as tile
from concourse.bass_utils import run_bass_kernel_spmd
from concourse.masks import make_identity

FP = mybir.dt.float32
BF = mybir.dt.bfloat16
NPBF = ml_dtypes.bfloat16
AF = mybir.ActivationFunctionType
ALU = mybir.AluOpType

NCORES = 8
B = 16384
BC = B // NCORES          # samples per core
IN, H, D = 512, 256, 128
NEXP = 14                 # 4 shared + 6 domain + 2*2 task experts
NE = 12                   # experts seen by each task gate
T = 2
GH = 64
TH = 64
TILE = 512                # samples per on-chip tile
NTILES = BC // TILE       # 4
SC = TILE // 128          # sample chunks per tile
KC = IN // 128            # contraction chunks for L1
HC = H // 128             # contraction chunks for L2

_CACHE = {}
# matmul lhsT/rhs at SBUF partition offset 64 compiles but faults at runtime
# on this toolchain, so gate L2 copies each task's operands to partition 0.
STACK_GATE_L2 = bool(int(os.environ.get("STACK_GATE_L2", "0")))


def _tasks_of_expert(e):
    """(task, gate_column) pairs that expert e feeds."""
    if e < 10:
        return [(0, e), (1, e)]
    if e < 12:
        return [(0, e)]          # task-0 experts -> gate cols 10, 11
    return [(1, e - 2)]          # task-1 experts -> gate cols 10, 11


def _split_excess_waits(nc, cap=1):
    """walrus's per-engine instruction structs carry a single sync-wait
    command; any scheduled instruction with >1 waits fails codegen ("Too many
    sync wait commands").  Move excess waits onto NoOp instructions inserted
    right before the offending instruction on the same engine."""
    for b in nc.m.functions[0].blocks:
        out = []
        changed = False
        for ins in b.instructions:
            si = getattr(ins, "sync_info", None)
            if si is not None and len(si.on_wait) > cap:
                extra = si.on_wait[:-cap]
                for j, w in enumerate(extra):
                    out.append(mybir.InstNoOp(
                        name=f"{ins.name}-wsplit{j}",
                        engine=ins.engine,
                        ins=[], outs=[],
                        sync_info=mybir.SyncInfo(on_wait=[w], on_update=[]),
                    ))
                ins.sync_info = mybir.SyncInfo(
                    on_wait=si.on_wait[-cap:], on_update=si.on_update)
                changed = True
            out.append(ins)
        if changed:
            b.instructions = out


def _build_bass(apply_bias=True):
    nc = bass.Bass(trn_type="TRN2", target_bir_lowering=False)

    xT = nc.dram_tensor("xT", [IN, BC], BF, kind="ExternalInput")
    w1 = nc.dram_tensor("w1", [128, NEXP * KC * HC * 128], BF, kind="ExternalInput")
    w2 = nc.dram_tensor("w2", [128, NEXP * HC * 128], BF, kind="ExternalInput")
    b1 = nc.dram_tensor("b1", [128, HC * NEXP], FP, kind="ExternalInput")
    b2 = nc.dram_tensor("b2", [1, NEXP * TILE], BF, kind="ExternalInput")
    gw1 = nc.dram_tensor("gw1", [128, KC * T * GH], BF, kind="ExternalInput")
    gb1 = nc.dram_tensor("gb1", [T * GH, 1], FP, kind="ExternalInput")
    gw2 = nc.dram_tensor("gw2", [T * GH, NE], BF, kind="ExternalInput")
    gb2 = nc.dram_tensor("gb2", [1, T * NE], BF, kind="ExternalInput")
    ow1 = nc.dram_tensor("ow1", [D, T * TH], BF, kind="ExternalInput")
    ob1 = nc.dram_tensor("ob1", [TH, T], FP, kind="ExternalInput")
    ow2 = nc.dram_tensor("ow2", [TH, T], BF, kind="ExternalInput")
    ob2h = nc.dram_tensor("ob2h", [1, T], FP, kind="ExternalInput")
    out = nc.dram_tensor("out", [1, BC * T], FP, kind="ExternalOutput")

    with tile.TileContext(nc) as tc, ExitStack() as ctx:
        wpool = ctx.enter_context(tc.tile_pool(name="weights", bufs=1))
        xpool = ctx.enter_context(tc.tile_pool(name="x", bufs=2))
        hpool = ctx.enter_context(tc.tile_pool(name="h", bufs=2))
        eopool = ctx.enter_context(tc.tile_pool(name="eo", bufs=3))
        combpool = ctx.enter_context(tc.tile_pool(name="comb", bufs=2))
        gpool = ctx.enter_context(tc.tile_pool(name="g", bufs=2))
        tpool = ctx.enter_context(tc.tile_pool(name="tower", bufs=2))
        opool = ctx.enter_context(tc.tile_pool(name="outrow", bufs=2))
        psA = ctx.enter_context(tc.tile_pool(name="psA", bufs=3, space=bass.MemorySpace.PSUM))
        psB = ctx.enter_context(tc.tile_pool(name="psB", bufs=2, space=bass.MemorySpace.PSUM))
        psC = ctx.enter_context(tc.tile_pool(name="psC", bufs=2, space=bass.MemorySpace.PSUM))
        psD = ctx.enter_context(tc.tile_pool(name="psD", bufs=1, space=bass.MemorySpace.PSUM))

        # ---- resident weights -------------------------------------------
        # Small gate/tower weights + the first x tile first, so compute can
        # start while the bulk expert weights stream in per-expert.
        gw1_sb = wpool.tile([128, KC * T * GH], BF)
        nc.sync.dma_start(gw1_sb[:], gw1[:])
        gb1_sb = wpool.tile([T * GH, 1], FP)
        nc.sync.dma_start(gb1_sb[:], gb1[:])
        gw2_sb = wpool.tile([T * GH, NE], BF)
        nc.sync.dma_start(gw2_sb[:], gw2[:])
        gb2_sb = wpool.tile([1, T * NE], BF)
        nc.sync.dma_start(gb2_sb[:], gb2[:])
        b1_sb = wpool.tile([128, HC * NEXP], FP)
        nc.sync.dma_start(b1_sb[:], b1[:])
        b2_sb = wpool.tile([1, NEXP * TILE], BF)
        nc.sync.dma_start(b2_sb[:], b2[:])

        w1_sb = wpool.tile([128, NEXP * KC * HC * 128], BF)
        w2_sb = wpool.tile([128, NEXP * HC * 128], BF)
        for e in range(NEXP):
            o = e * KC * HC * 128
            nc.sync.dma_start(w1_sb[:, o:o + KC * HC * 128],
                              w1[:, o:o + KC * HC * 128])
            o2 = e * HC * 128
            nc.sync.dma_start(w2_sb[:, o2:o2 + HC * 128],
                              w2[:, o2:o2 + HC * 128])

        ow1_sb = wpool.tile([D, T * TH], BF)
        nc.sync.dma_start(ow1_sb[:], ow1[:])
        ob1_sb = wpool.tile([TH, T], FP)
        nc.sync.dma_start(ob1_sb[:], ob1[:])
        ow2_sb = wpool.tile([TH, T], BF)
        nc.sync.dma_start(ow2_sb[:], ow2[:])
        ob2h_sb = wpool.tile([1, T], FP)
        nc.sync.dma_start(ob2h_sb[:], ob2h[:])

        ident = wpool.tile([128, 128], FP)
        make_identity(nc, ident[:])
        ones_sb = wpool.tile([1, 128], BF)
        nc.vector.memset(ones_sb[:], 1.0)
        # touch the ACT table set before the critical path: the implicit
        # ACT_TABLE_LOAD (~1.3us) then overlaps the initial weight DMAs
        actwarm = wpool.tile([1, 8], FP)
        nc.vector.memset(actwarm[:], 0.0)
        nc.scalar.activation(actwarm[:], actwarm[:], AF.Exp)

        def w1_sl(e, kc, hc):
            o = ((e * KC + kc) * HC + hc) * 128
            return w1_sb[:, o:o + 128]

        def w2_sl(e, hc):
            o = (e * HC + hc) * 128
            return w2_sb[:, o:o + 128]

        # ---- per-tile pipeline ------------------------------------------
        for i in range(NTILES):
            xt = []
            for kc in range(KC):
                t_ = xpool.tile([128, TILE], BF, tag=f"xt{kc}")
                # gpsimd SWDGE ring: x tiles must not queue behind the bulk
                # weight stream on the sync HWDGE ring
                nc.gpsimd.dma_start(
                    t_[:], xT[kc * 128:(kc + 1) * 128, i * TILE:(i + 1) * TILE])
                xt.append(t_)

            # ---------------- gates ----------------
            # both tasks' gate hiddens stacked on the partition axis (2*64)
            gps = psC.tile([T * GH, TILE], FP, tag="gate")
            for kc in range(KC):
                nc.tensor.matmul(gps[:], gw1_sb[:, kc * 128:(kc + 1) * 128],
                                 xt[kc][:],
                                 start=(kc == 0), stop=(kc == KC - 1))
            if STACK_GATE_L2:
                g1 = gpool.tile([T * GH, TILE], BF, tag="g1")
                nc.scalar.activation(g1[:], gps[:], AF.Relu, bias=gb1_sb[:])
                g1s = [g1[t * GH:(t + 1) * GH, :] for t in range(T)]
                gw2s = [gw2_sb[t * GH:(t + 1) * GH, :] for t in range(T)]
            else:
                g1s, gw2s = [], []
                for t in range(T):
                    g1t = gpool.tile([GH, TILE], BF, tag=f"g1_{t}", name=f"g1_{t}")
                    nc.scalar.activation(g1t[:], gps[t * GH:(t + 1) * GH, :],
                                         AF.Relu, bias=gb1_sb[t * GH:(t + 1) * GH, :])
                    g1s.append(g1t[:, :])
                    gw2t = gpool.tile([GH, NE], BF, tag=f"gw2_{t}", name=f"gw2_{t}")
                    nc.vector.tensor_copy(gw2t[:], gw2_sb[t * GH:(t + 1) * GH, :])
                    gw2s.append(gw2t[:, :])

            lps = psD.tile([128, SC * T * NE], FP, tag="small")
            for sc in range(SC):
                o = sc * (T * NE)
                if apply_bias:
                    nc.tensor.matmul(lps[:, o:o + T * NE], ones_sb[:], gb2_sb[:],
                                     start=True, stop=False, skip_group_check=True)
                for t in range(T):
                    nc.tensor.matmul(lps[:, o + t * NE:o + (t + 1) * NE],
                                     g1s[t][:, sc * 128:(sc + 1) * 128],
                                     gw2s[t],
                                     start=not apply_bias,
                                     stop=(t == T - 1) if apply_bias else True,
                                     skip_group_check=True)

            exp_sb = gpool.tile([128, SC * T * NE], FP, tag="exp")
            nc.scalar.activation(exp_sb[:], lps[:], AF.Exp)
            sums = gpool.tile([128, SC * T], FP, tag="gsum")
            nc.vector.tensor_reduce(
                sums[:],
                exp_sb[:].rearrange("p (g e) -> p g e", e=NE),
                axis=mybir.AxisListType.X, op=ALU.add)
            rec = gpool.tile([128, SC * T], FP, tag="grec")
            nc.vector.reciprocal(rec[:], sums[:])
            g_sb = gpool.tile([128, SC * T * NE], FP, tag="g")
            for idx in range(SC * T):
                nc.vector.tensor_scalar_mul(
                    g_sb[:, idx * NE:(idx + 1) * NE],
                    exp_sb[:, idx * NE:(idx + 1) * NE],
                    rec[:, idx:idx + 1])

            # ---------------- experts + combine ----------------
            comb = [combpool.tile([128, TILE], FP, tag=f"comb{t}", name=f"comb{t}")
                    for t in range(T)]
            comb_started = [False] * T

            for e in range(NEXP):
                h_tiles = []
                for hc in range(HC):
                    l1ps = psA.tile([128, TILE], FP, tag="l1")
                    for kc in range(KC):
                        nc.tensor.matmul(l1ps[:], w1_sl(e, kc, hc), xt[kc][:],
                                         start=(kc == 0), stop=(kc == KC - 1))
                    ht = hpool.tile([128, TILE], BF, tag=f"h{hc}")
                    nc.scalar.activation(ht[:], l1ps[:], AF.Relu,
                                         bias=b1_sb[:, hc * NEXP + e:hc * NEXP + e + 1])
                    h_tiles.append(ht)

                l2ps = psB.tile([128, TILE], FP, tag="l2")
                if apply_bias:
                    nc.tensor.matmul(l2ps[:], ones_sb[:],
                                     b2_sb[:, e * TILE:(e + 1) * TILE],
                                     start=True, stop=False, skip_group_check=True)
                for sc in range(SC):
                    sl = l2ps[:, sc * 128:(sc + 1) * 128]
                    for hc in range(HC):
                        nc.tensor.matmul(sl, h_tiles[hc][:, sc * 128:(sc + 1) * 128],
                                         w2_sl(e, hc),
                                         start=(hc == 0) and not apply_bias,
                                         stop=(hc == HC - 1) if not apply_bias
                                              else (sc == SC - 1 and hc == HC - 1),
                                         skip_group_check=True)
                eo = eopool.tile([128, TILE], BF, tag="eo")
                nc.scalar.activation(eo[:], l2ps[:], AF.Relu)

                for (t, gcol) in _tasks_of_expert(e):
                    eng = nc.vector
                    for sc in range(SC):
                        c_sl = comb[t][:, sc * 128:(sc + 1) * 128]
                        e_sl = eo[:, sc * 128:(sc + 1) * 128]
                        g_ap = g_sb[:, sc * (T * NE) + t * NE + gcol:
                                    sc * (T * NE) + t * NE + gcol + 1]
                        if not comb_started[t]:
                            eng.tensor_scalar_mul(c_sl, e_sl, g_ap)
                        else:
                            eng.scalar_tensor_tensor(
                                c_sl, e_sl, g_ap, c_sl, op0=ALU.mult, op1=ALU.add)
                    comb_started[t] = True

            # ---------------- towers ----------------
            orow = opool.tile([1, TILE, T], FP, tag="orow")
            for t in range(T):
                trps = psB.tile([128, TILE], FP, tag="l2")
                for sc in range(SC):
                    nc.tensor.transpose(trps[:, sc * 128:(sc + 1) * 128],
                                        comb[t][:, sc * 128:(sc + 1) * 128],
                                        ident[:])
                combT = tpool.tile([128, TILE], BF, tag="combT")
                nc.scalar.copy(combT[:], trps[:])
                t1ps = psC.tile([TH, TILE], FP, tag="gate")
                nc.tensor.matmul(t1ps[:], ow1_sb[:, t * TH:(t + 1) * TH], combT[:],
                                 start=True, stop=True)
                th = tpool.tile([TH, TILE], BF, tag="th")
                nc.scalar.activation(th[:], t1ps[:], AF.Relu,
                                     bias=ob1_sb[:, t:t + 1])
                t2ps = psD.tile([1, TILE], FP, tag="small")
                nc.tensor.matmul(t2ps[:], ow2_sb[:, t:t + 1], th[:],
                                 start=True, stop=True)
                tnh = opool.tile([1, TILE], FP, tag=f"tanh{t}")
                nc.scalar.activation(tnh[:], t2ps[:], AF.Tanh,
                                     scale=0.5, bias=ob2h_sb[:, t:t + 1])
                # 0.5*tanh + 0.5 = sigmoid, interleaved into [s, T] order
                nc.vector.tensor_scalar(
                    orow[:, :, t], tnh[:], 0.5, 0.5,
                    op0=ALU.mult, op1=ALU.add)

            nc.sync.dma_start(
                out[:, i * TILE * T:(i + 1) * TILE * T],
                orow[:].rearrange("p a b -> p (a b)"))

    _split_excess_waits(nc)
    return nc


def _pack_inputs(x, sw1, sb1, sw2, sb2, dw1, db1, dw2, db2,
                 tw1, tb1, tw2, tb2, gw1, gb1, gw2, gb2,
                 ow1, ob1, ow2, ob2):
    f = np.float32
    w1_all = np.concatenate([sw1, dw1, tw1.reshape(T * 2, IN, H)], 0).astype(f)
    w2_all = np.concatenate([sw2, dw2, tw2.reshape(T * 2, H, D)], 0).astype(f)
    b1_all = np.concatenate([sb1, db1, tb1.reshape(T * 2, H)], 0).astype(f)
    b2_all = np.concatenate([sb2, db2, tb2.reshape(T * 2, D)], 0).astype(f)

    common = {
        "w1": np.ascontiguousarray(
            w1_all.reshape(NEXP, KC, 128, HC, 128).transpose(2, 0, 1, 3, 4)
            .reshape(128, NEXP * KC * HC * 128)).astype(NPBF),
        "w2": np.ascontiguousarray(
            w2_all.reshape(NEXP, HC, 128, 128).transpose(2, 0, 1, 3)
            .reshape(128, NEXP * HC * 128)).astype(NPBF),
        "b1": np.ascontiguousarray(
            b1_all.reshape(NEXP, HC, 128).transpose(2, 1, 0).reshape(128, HC * NEXP)),
        "b2": np.ascontiguousarray(
            np.tile(b2_all[:, None, :], (1, SC, 1)).reshape(1, NEXP * TILE)
        ).astype(NPBF),
        "gw1": np.ascontiguousarray(
            gw1.reshape(T, KC, 128, GH).transpose(2, 1, 0, 3)
            .reshape(128, KC * T * GH)).astype(NPBF),
        "gb1": np.ascontiguousarray(gb1.reshape(T * GH, 1).astype(f)),
        "gw2": np.ascontiguousarray(gw2.reshape(T * GH, NE)).astype(NPBF),
        "gb2": np.ascontiguousarray(gb2.reshape(1, T * NE)).astype(NPBF),
        "ow1": np.ascontiguousarray(
            ow1.transpose(1, 0, 2).reshape(D, T * TH)).astype(NPBF),
        "ob1": np.ascontiguousarray(ob1.T.astype(f)),
        "ow2": np.ascontiguousarray(
            ow2.transpose(1, 0, 2).reshape(TH, T)).astype(NPBF),
        "ob2h": np.ascontiguousarray((0.5 * ob2).reshape(1, T).astype(f)),
    }

    xTfull = np.ascontiguousarray(x.astype(f).T.astype(NPBF))   # [IN, B]
    in_maps = []
    for c in range(NCORES):
        m = dict(common)
        m["xT"] = np.ascontiguousarray(xTfull[:, c * BC:(c + 1) * BC])
        in_maps.append(m)
    return in_maps


def kernel(**inputs):
    inputs = {k: np.asarray(v) for k, v in inputs.items()}
    inputs.pop("domain_ids", None)   # unused by the reference computation
    x = inputs.pop("x")

    zero_bias = all(
        not np.any(np.asarray(inputs[k]))
        for k in ("sb2", "db2", "tb2", "gb2"))
    key = ("nc", not zero_bias)
    if key not in _CACHE:
        _CACHE[key] = _build_bass(apply_bias=not zero_bias)
    nc = _CACHE[key]

    in_maps = _pack_inputs(x=x, **inputs)

    trace = bool(int(os.environ.get("KERNEL_TRACE", "0")))
    res = run_bass_kernel_spmd(nc, in_maps, core_ids=list(range(NCORES)),
                               trace=trace)
    _CACHE["last_results"] = res

    out = np.concatenate(
        [res.results[c]["out"].reshape(BC, T) for c in range(NCORES)], axis=0)
    return out.astype(np.float32)
